# revision 24
# baseline (speedup 1.0000x reference)
"""Trainium2 Bass kernel for nn_AudNet (4-layer LIF SNN, 81-step scan).

Strategy (per core, batch 128 of 1024; data-parallel over 8 cores):
  - Layout: features on partitions, batch on the free dim.
  - Timesteps processed in pairs so every matmul has free dim 256, where
    fp32r runs at 1 cycle/row on the PE.
  - All weights split into fp32r hi + lo terms (residual ~2^-26), so the
    matmuls match fp32 numerics to ~1e-8.  x is split hi/lo too (3-term
    layer 1; the hi/lo split of x is exact).
  - LIF: reset(t) == spk(t-1), so
        mem = 0.95*mem + (cur + b)        (A: scalar_tensor_tensor, PSUM in)
        mem = mem - spk_prev              (B: tensor_sub, in place)
        spk = mem > 1                     (C: tensor_scalar is_gt -> fp32r)
    Biases ride inside the matmuls via constant-one rows in padding
    partitions of the stationary tiles.
  - Outputs (spk4/mem4, [10,128] per step) are PE-transposed into a PSUM
    accumulator and evacuated to SBUF every 24 steps; two strided DMAs
    write the [81,128,10] DRAM outputs.
"""

import numpy as np

import concourse.bass as bass
import concourse.mybir as mybir
import concourse.tile as tile
from concourse.bass_utils import run_bass_kernel_spmd

F32 = mybir.dt.float32
# Reduced dtype for all matmul operands.  fp16 (11-bit mantissa) hi/lo
# 2-term splits carry ~2^-24 residual -- near-fp32 -- while the PE
# streams fp16 moving data at 1 cycle/row (fp32r runs at ~2) and fp16
# stationary tiles get Fast Weight Load (fp32r cannot).
F32R = mybir.dt.float16
GT = mybir.AluOpType.is_gt
MULT = mybir.AluOpType.mult
ADD = mybir.AluOpType.add

T_FULL = 81
F = 129          # input features per step
H = 1000         # hidden width
HP = 1024        # padded hidden width (8 chunks of 128)
HL = 20          # layer-3 width
O = 10           # output width
B = 128          # batch per core
NCORES = 8
BETA = 0.95
TH = 1.0


def fix_multi_waits(nc, limit=1):
    """walrus codegen rejects >1 sem wait on most instructions; spill
    extras onto standalone EventSemaphore instructions in front."""
    ev = 0
    for bb in nc.main_func.blocks:
        out = []
        for ins in bb.instructions:
            si = ins.sync_info
            if si is not None and len(si.on_wait) > limit:
                waits = list(si.on_wait)
                extra, keep = waits[:-limit], waits[-limit:]
                for w in extra:
                    e = mybir.InstEventSemaphore(name=f"evw_{ev}", ins=[], outs=[])
                    ev += 1
                    e.engine = ins.engine
                    e.sync_info = mybir.SyncInfo(on_wait=[w], on_update=[])
                    out.append(e)
                ins.sync_info = mybir.SyncInfo(on_wait=keep, on_update=list(si.on_update))
            out.append(ins)
        bb.instructions = out


def build_nc(T=T_FULL, debug=False, rdt=None):
    global F32R
    prev_r = F32R
    if rdt is not None:
        F32R = rdt
    NPAIR = (T + 1) // 2
    last_odd = (T % 2) == 1  # final pair has only one real step

    nc = bass.Bass()
    _four = mybir.dt.size(F32R) == 4

    def rd(ap):
        """view a reduced-dtype AP as f32 for DVE arithmetic"""
        return ap.bitcast(F32) if _four else ap

    x_d = nc.declare_dram_parameter("x", [B, F * T_FULL], F32, isOutput=False)
    W1_d = nc.declare_dram_parameter("W1", [H, F], F32, isOutput=False)
    b1_d = nc.declare_dram_parameter("b1", [H], F32, isOutput=False)
    W2_d = nc.declare_dram_parameter("W2", [H, H], F32, isOutput=False)
    b2_d = nc.declare_dram_parameter("b2", [H], F32, isOutput=False)
    W3_d = nc.declare_dram_parameter("W3", [HL, H], F32, isOutput=False)
    b3_d = nc.declare_dram_parameter("b3", [HL], F32, isOutput=False)
    W4_d = nc.declare_dram_parameter("W4", [O, HL], F32, isOutput=False)
    b4_d = nc.declare_dram_parameter("b4", [O], F32, isOutput=False)
    eye_d = nc.declare_dram_parameter("eye", [128, 128], F32, isOutput=False)
    if debug:
        dbg = {
            "mem1_out": nc.declare_dram_parameter("mem1_out", [128, HP], F32, isOutput=True),
            "mem2_out": nc.declare_dram_parameter("mem2_out", [128, HP], F32, isOutput=True),
            "mem3_out": nc.declare_dram_parameter("mem3_out", [HL, B], F32, isOutput=True),
            "spk1_out": nc.declare_dram_parameter("spk1_out", [128, 2 * HP], F32, isOutput=True),
            "spk2_out": nc.declare_dram_parameter("spk2_out", [128, 2 * HP], F32, isOutput=True),
        }
    spk_o = nc.declare_dram_parameter("spk_out", [T, B, O], F32, isOutput=True)
    mem_o = nc.declare_dram_parameter("mem_out", [T, B, O], F32, isOutput=True)

    with tile.TileContext(nc) as tc:
        with tc.tile_pool(name="persist", bufs=1) as pp:
            # ---- persistent SBUF tiles ----
            eye = pp.tile([128, 128], F32, tag="eye")
            xT = pp.tile([128, 128 * T_FULL], F32, tag="xT")      # [f<128, b*T+t]
            w1h = pp.tile([128, HP], F32R, tag="w1h")
            w1l = pp.tile([128, HP], F32R, tag="w1l")
            w1bh = pp.tile([2, HP], F32R, tag="w1bh")             # row0 f=128, row1 bias
            w1bl = pp.tile([2, HP], F32R, tag="w1bl")
            w2h = [pp.tile([128, HP], F32R, tag=f"w2h{c}", name=f"w2h{c}") for c in range(8)]
            w2l = [pp.tile([128, HP], F32R, tag=f"w2l{c}", name=f"w2l{c}") for c in range(8)]
            w3h = pp.tile([128, 8 * HL], F32R, tag="w3h")
            w3l = pp.tile([128, 8 * HL], F32R, tag="w3l")
            w4h = pp.tile([HL + 1, O], F32R, tag="w4h")
            w4l = pp.tile([HL + 1, O], F32R, tag="w4l")
            mem1 = pp.tile([128, HP], F32, tag="mem1")
            mem2 = pp.tile([128, HP], F32, tag="mem2")
            mem3 = pp.tile([HL, B], F32, tag="mem3")
            mem4 = pp.tile([O, 2 * B], F32, tag="mem4")
            spk1 = pp.tile([128, 2 * HP], F32R, tag="spk1")       # slot-major
            spk2 = pp.tile([128, 2 * HP], F32R, tag="spk2")
            spk3 = pp.tile([HL + 1, 2 * B], F32R, tag="spk3")     # row HL = ones
            spk4 = pp.tile([O, 2 * B], F32, tag="spk4")
            outbuf = pp.tile([128, T * 2 * O], F32, tag="outbuf")
            ones_f = pp.tile([1, 256], F32, tag="ones_f")
            ones_r = pp.tile([1, 256], F32R, tag="ones_r")

            nc.sync.dma_start(out=eye[:], in_=eye_d[:])

            # ================= SETUP =================
            with (
                tc.tile_pool(name="setup_sb", bufs=1) as sp,
                tc.tile_pool(name="setup_ps", bufs=4, space="PSUM") as spp,
            ):
                # bias splits (hi/lo in fp32r), kept in partition 0
                def bias_split(b_dram, n, tagbase):
                    bs = sp.tile([1, n], F32, tag=f"{tagbase}s")
                    nc.sync.dma_start(out=bs[:], in_=b_dram[:].rearrange("(a n) -> a n", a=1))
                    bh = sp.tile([1, n], F32R, tag=f"{tagbase}h")
                    bl = sp.tile([1, n], F32R, tag=f"{tagbase}l")
                    nc.vector.tensor_copy(out=bh[:], in_=bs[:])
                    nc.vector.tensor_sub(bl[:], bs[:], rd(bh[:]))
                    return bh, bl

                b1h, b1l = bias_split(b1_d, H, "b1")
                b2h, b2l = bias_split(b2_d, H, "b2")
                b3h, b3l = bias_split(b3_d, HL, "b3")
                b4h, b4l = bias_split(b4_d, O, "b4")

                # zero-init all weight tiles (padding regions stay 0)
                for tl in [w1h, w1l, w1bh, w1bl, w3h, w3l, w4h, w4l] + w2h + w2l:
                    nc.vector.memset(rd(tl[:]), 0.0)

                def evac_split(psum_ap, hi_ap, lo_ap):
                    nc.vector.tensor_copy(out=hi_ap, in_=psum_ap)
                    nc.vector.tensor_sub(lo_ap, psum_ap, rd(hi_ap))

                # ---- W2 ----
                for mc in range(8):
                    mh = 128 if mc < 7 else H - 7 * 128
                    ws = sp.tile([128, H], F32, tag="w2s", bufs=2)
                    nc.sync.dma_start(out=ws[:mh, :], in_=W2_d[mc * 128 : mc * 128 + mh, :])
                    for c in range(8):
                        kw = 128 if c < 7 else H - 7 * 128
                        pt = spp.tile([128, 128], F32, tag="tp")
                        nc.tensor.transpose(
                            pt[:kw, :mh], ws[:mh, c * 128 : c * 128 + kw], eye[:mh, :mh]
                        )
                        evac_split(
                            pt[:kw, :mh],
                            w2h[c][:kw, mc * 128 : mc * 128 + mh],
                            w2l[c][:kw, mc * 128 : mc * 128 + mh],
                        )
                # bias rows: k-chunk 7, partition 104 (feature 1000)
                nc.sync.dma_start(out=w2h[7][104:105, 0:H], in_=b2h[:])
                nc.sync.dma_start(out=w2l[7][104:105, 0:H], in_=b2l[:])

                # ---- W1 ----
                for mc in range(8):
                    mh = 128 if mc < 7 else H - 7 * 128
                    ws = sp.tile([128, F], F32, tag="w1s")
                    nc.sync.dma_start(out=ws[:mh, :], in_=W1_d[mc * 128 : mc * 128 + mh, :])
                    pt = spp.tile([128, 128], F32, tag="tp")
                    nc.tensor.transpose(pt[:128, :mh], ws[:mh, 0:128], eye[:mh, :mh])
                    evac_split(
                        pt[:128, :mh],
                        w1h[:, mc * 128 : mc * 128 + mh],
                        w1l[:, mc * 128 : mc * 128 + mh],
                    )
                    pt2 = spp.tile([128, 128], F32, tag="tp2")
                    nc.tensor.transpose(pt2[:1, :mh], ws[:mh, 128:129], eye[:mh, :mh])
                    evac_split(
                        pt2[:1, :mh],
                        w1bh[0:1, mc * 128 : mc * 128 + mh],
                        w1bl[0:1, mc * 128 : mc * 128 + mh],
                    )
                nc.sync.dma_start(out=w1bh[1:2, 0:H], in_=b1h[:])
                nc.sync.dma_start(out=w1bl[1:2, 0:H], in_=b1l[:])

                # ---- W3 ----
                w3s = sp.tile([HL, H], F32, tag="w3s")
                nc.sync.dma_start(out=w3s[:], in_=W3_d[:])
                for c in range(8):
                    kw = 128 if c < 7 else H - 7 * 128
                    pt = spp.tile([128, 128], F32, tag="tp")
                    nc.tensor.transpose(
                        pt[:kw, :HL], w3s[:, c * 128 : c * 128 + kw], eye[:HL, :HL]
                    )
                    evac_split(
                        pt[:kw, :HL],
                        w3h[:kw, c * HL : (c + 1) * HL],
                        w3l[:kw, c * HL : (c + 1) * HL],
                    )
                nc.sync.dma_start(out=w3h[104:105, 7 * HL : 8 * HL], in_=b3h[:])
                nc.sync.dma_start(out=w3l[104:105, 7 * HL : 8 * HL], in_=b3l[:])

                # ---- W4 ----
                w4s = sp.tile([O, HL], F32, tag="w4s")
                nc.sync.dma_start(out=w4s[:], in_=W4_d[:])
                pt = spp.tile([128, 128], F32, tag="tp")
                nc.tensor.transpose(pt[:HL, :O], w4s[:, :], eye[:O, :O])
                evac_split(pt[:HL, :O], w4h[:HL, :], w4l[:HL, :])
                nc.sync.dma_start(out=w4h[HL : HL + 1, :], in_=b4h[:])
                nc.sync.dma_start(out=w4l[HL : HL + 1, :], in_=b4l[:])

                # ---- x (transposed load: [f, b, t]) ----
                xv = x_d[:].rearrange("b (f t) -> f b t", t=T_FULL)
                nc.sync.dma_start(
                    out=xT[:].rearrange("p (b t) -> p b t", t=T_FULL),
                    in_=xv[0:128, :, :],
                )

                # ---- state init ----
                for tl in [mem1, mem2, mem3, mem4, spk4]:
                    nc.vector.memset(tl[:], 0.0)
                nc.vector.memset(rd(spk1[:]), 0.0)
                nc.vector.memset(rd(spk2[:]), 0.0)
                nc.vector.memset(rd(spk3[:]), 0.0)
                # constant-one bias rhs rows (both slots).  Compute
                # engines need 32-aligned partition starts, so write these
                # single rows via DMA from a ones tile.
                nc.vector.memset(ones_f[:], 1.0)
                nc.vector.tensor_copy(out=ones_r[:], in_=ones_f[:])
                s1w = spk1[:].rearrange("p (s c b) -> p s c b", s=2, b=B)
                s2w = spk2[:].rearrange("p (s c b) -> p s c b", s=2, b=B)
                ones_v = ones_r[:].rearrange("p (s b) -> p s b", s=2)
                nc.sync.dma_start(out=s1w[104:105, :, 7, :], in_=ones_v)
                nc.sync.dma_start(out=s2w[104:105, :, 7, :], in_=ones_v)
                nc.sync.dma_start(out=spk3[HL : HL + 1, :], in_=ones_r[:])

            # ================= SCAN =================
            with (
                tc.tile_pool(name="xs", bufs=2) as xsp,
                tc.tile_pool(name="xb", bufs=3) as xbp,
                tc.tile_pool(name="pl1", bufs=1, space="PSUM") as pl1,
                tc.tile_pool(name="pl2", bufs=1, space="PSUM") as pl2,
                tc.tile_pool(name="pl34", bufs=1, space="PSUM") as pl34,
                tc.tile_pool(name="pout", bufs=1, space="PSUM") as pout,
            ):
                outacc = pout.tile([128, 24 * 2 * O], F32, tag="outacc")
                evac = {"done": 0}

                xT_v = xT[:].rearrange("p (b t) -> p t b", t=T_FULL)
                xb_dram = x_d[:].rearrange("b (f t) -> f t b", t=T_FULL)[128:129]
                s1_v = spk1[:].rearrange("p (s c b) -> p s c b", s=2, b=B)
                s2_v = spk2[:].rearrange("p (s c b) -> p s c b", s=2, b=B)
                m1_v = mem1[:].rearrange("p (c b) -> p c b", b=B)
                m2_v = mem2[:].rearrange("p (c b) -> p c b", b=B)

                xb_tiles = {}

                def dma_xb(p):
                    """prefetch the f=128 feature row for pair p; this DMA is
                    a slow strided gather, so it is issued pairs ahead."""
                    t0 = 2 * p
                    nt = 1 if (last_odd and p == NPAIR - 1) else 2
                    xbr = xbp.tile([2, 256], F32, tag="xbr")
                    xbr_w = xbr[:].rearrange("p (t b) -> p t b", b=B)
                    nc.sync.dma_start(
                        out=xbr_w[0:1, 0:nt, :], in_=xb_dram[:, t0 : t0 + nt, :]
                    )
                    if p < 3:
                        nc.sync.dma_start(out=xbr[1:2, :], in_=ones_f[:])
                    xb_tiles[p] = xbr

                def split_x(p):
                    """fp16 hi/lo split of the x slice for pair p (t-major
                    pair columns); last odd pair duplicates its single step."""
                    t0 = 2 * p
                    dup = last_odd and p == NPAIR - 1
                    nt = 1 if dup else 2
                    src = xT_v[:, t0 : t0 + nt, :]
                    xh = xsp.tile([128, 256], F32R, tag="xh")
                    xl = xsp.tile([128, 256], F32R, tag="xl")
                    xbh = xsp.tile([2, 256], F32R, tag="xbh")
                    xbl = xsp.tile([2, 256], F32R, tag="xbl")
                    xbr = xb_tiles.pop(p)
                    xbr_w = xbr[:].rearrange("p (t b) -> p t b", b=B)
                    if dup:
                        nc.vector.tensor_copy(
                            out=xbr_w[0:1, 1:2, :], in_=xbr_w[0:1, 0:1, :]
                        )
                    srcb = xbr_w[:, 0:2, :]
                    xh_w = xh[:].rearrange("p (t b) -> p t b", b=B)
                    xl_w = xl[:].rearrange("p (t b) -> p t b", b=B)
                    xbh_w = xbh[:].rearrange("p (t b) -> p t b", b=B)
                    xbl_w = xbl[:].rearrange("p (t b) -> p t b", b=B)
                    for s in range(2 if dup else 1):
                        dst = slice(s, s + nt) if dup else slice(0, 2)
                        nc.scalar.copy(out=xh_w[:, dst, :], in_=src)
                        nc.vector.tensor_sub(
                            xl_w[:, dst, :], src, rd(xh_w[:, dst, :])
                        )
                    nc.scalar.copy(out=xbh_w[:], in_=srcb)
                    nc.vector.tensor_sub(xbl_w[:], srcb, rd(xbh_w[:]))
                    return xh, xl, xbh, xbl

                def mm1(p, xs):
                    xh, xl, xbh, xbl = xs
                    t1 = pl1.tile([128, 2048], F32, tag="l1")
                    for mc in range(8):
                        out = t1[:, mc * 256 : (mc + 1) * 256]
                        ms = slice(mc * 128, (mc + 1) * 128)
                        terms = [
                            (w1h[:, ms], xh[:]),
                            (w1bh[:, ms], xbh[:]),
                            (w1h[:, ms], xl[:]),
                            (w1bh[:, ms], xbl[:]),
                            (w1l[:, ms], xh[:]),
                            (w1bl[:, ms], xbh[:]),
                        ]
                        for i, (lhsT, rhs) in enumerate(terms):
                            nc.tensor.matmul(
                                out, lhsT=lhsT, rhs=rhs,
                                start=(i == 0), stop=(i == len(terms) - 1),
                            )
                    return t1

                def lif1(p, t1):
                    t0 = 2 * p
                    c1v = t1[:].rearrange("p (c s b) -> p s c b", s=2, b=B)
                    for s in range(2):
                        t = t0 + s
                        if t >= T:
                            break
                        sl, slp = t % 2, (t - 1) % 2
                        nc.vector.scalar_tensor_tensor(
                            out=m1_v[:, :, :], in0=m1_v[:, :, :], scalar=BETA,
                            in1=c1v[:, s, :, :], op0=MULT, op1=ADD,
                        )
                        nc.vector.tensor_sub(
                            mem1[:], mem1[:],
                            rd(spk1[:, slp * HP : (slp + 1) * HP]),
                        )
                        nc.vector.tensor_scalar(
                            out=s1_v[:, sl, 0:7, :], in0=m1_v[:, 0:7, :],
                            scalar1=TH, scalar2=None, op0=GT,
                        )
                        nc.vector.tensor_scalar(
                            out=s1_v[0:104, sl, 7, :], in0=m1_v[0:104, 7, :],
                            scalar1=TH, scalar2=None, op0=GT,
                        )

                def mm2_half(p, half):
                    t2 = pl2.tile([128, 1024], F32, tag="l2")
                    for mi in range(4):
                        mc = 4 * half + mi
                        out = t2[:, mi * 256 : (mi + 1) * 256]
                        n = 0
                        for wsp in (w2h, w2l):
                            for c in range(8):
                                nc.tensor.matmul(
                                    out,
                                    lhsT=wsp[c][:, mc * 128 : (mc + 1) * 128],
                                    rhs=s1_v[:, :, c, :],
                                    start=(n == 0), stop=(n == 15),
                                )
                                n += 1
                    return t2

                def lif2_half(p, half, t2):
                    t0 = 2 * p
                    c2v = t2[:].rearrange("p (c s b) -> p s c b", s=2, b=B)
                    c0 = 4 * half
                    for s in range(2):
                        t = t0 + s
                        if t >= T:
                            break
                        sl, slp = t % 2, (t - 1) % 2
                        nc.vector.scalar_tensor_tensor(
                            out=m2_v[:, c0 : c0 + 4, :], in0=m2_v[:, c0 : c0 + 4, :],
                            scalar=BETA, in1=c2v[:, s, :, :], op0=MULT, op1=ADD,
                        )
                        nc.vector.tensor_sub(
                            mem2[:, c0 * B : (c0 + 4) * B],
                            mem2[:, c0 * B : (c0 + 4) * B],
                            rd(spk2[:, slp * HP + c0 * B : slp * HP + (c0 + 4) * B]),
                        )
                        if half == 0:
                            nc.vector.tensor_scalar(
                                out=s2_v[:, sl, 0:4, :], in0=m2_v[:, 0:4, :],
                                scalar1=TH, scalar2=None, op0=GT,
                            )
                        else:
                            nc.vector.tensor_scalar(
                                out=s2_v[:, sl, 4:7, :], in0=m2_v[:, 4:7, :],
                                scalar1=TH, scalar2=None, op0=GT,
                            )
                            nc.vector.tensor_scalar(
                                out=s2_v[0:104, sl, 7, :], in0=m2_v[0:104, 7, :],
                                scalar1=TH, scalar2=None, op0=GT,
                            )

                def mm3(p):
                    t34 = pl34.tile([32, 512], F32, tag="l34")
                    out = t34[0:HL, 0:256]
                    n = 0
                    for wsp in (w3h, w3l):
                        for c in range(8):
                            nc.tensor.matmul(
                                out,
                                lhsT=wsp[:, c * HL : (c + 1) * HL],
                                rhs=s2_v[:, :, c, :],
                                start=(n == 0), stop=(n == 15),
                            )
                            n += 1
                    return t34

                def lif3(p, t34):
                    t0 = 2 * p
                    for s in range(2):
                        t = t0 + s
                        if t >= T:
                            break
                        sl, slp = t % 2, (t - 1) % 2
                        nc.vector.scalar_tensor_tensor(
                            out=mem3[:], in0=mem3[:], scalar=BETA,
                            in1=t34[0:HL, s * B : (s + 1) * B], op0=MULT, op1=ADD,
                        )
                        nc.vector.tensor_sub(
                            mem3[:], mem3[:],
                            rd(spk3[0:HL, slp * B : (slp + 1) * B]),
                        )
                        nc.vector.tensor_scalar(
                            out=spk3[0:HL, sl * B : (sl + 1) * B], in0=mem3[:],
                            scalar1=TH, scalar2=None, op0=GT,
                        )

                def mm4(p, t34):
                    out = t34[0:O, 256:512]
                    nc.tensor.matmul(out, lhsT=w4h[:], rhs=spk3[:], start=True, stop=False)
                    nc.tensor.matmul(out, lhsT=w4l[:], rhs=spk3[:], start=False, stop=True)

                def lif4(p, t34):
                    t0 = 2 * p
                    for s in range(2):
                        t = t0 + s
                        if t >= T:
                            break
                        sl, slp = t % 2, (t - 1) % 2
                        nc.vector.scalar_tensor_tensor(
                            out=mem4[:, sl * B : (sl + 1) * B],
                            in0=mem4[:, slp * B : (slp + 1) * B], scalar=BETA,
                            in1=t34[0:O, 256 + s * B : 256 + (s + 1) * B],
                            op0=MULT, op1=ADD,
                        )
                        nc.vector.tensor_sub(
                            mem4[:, sl * B : (sl + 1) * B],
                            mem4[:, sl * B : (sl + 1) * B],
                            spk4[:, slp * B : (slp + 1) * B],
                        )
                        nc.vector.tensor_scalar(
                            out=spk4[:, sl * B : (sl + 1) * B],
                            in0=mem4[:, sl * B : (sl + 1) * B],
                            scalar1=TH, scalar2=None, op0=GT,
                        )

                def record(p):
                    t0 = 2 * p
                    for s in range(2):
                        t = t0 + s
                        if t >= T:
                            break
                        sl = t % 2
                        w = t - evac["done"]
                        nc.tensor.transpose(
                            outacc[:, w * 2 * O : w * 2 * O + O],
                            spk4[:, sl * B : (sl + 1) * B],
                            eye[:O, :O],
                        )
                        nc.tensor.transpose(
                            outacc[:, w * 2 * O + O : (w + 1) * 2 * O],
                            mem4[:, sl * B : (sl + 1) * B],
                            eye[:O, :O],
                        )
                    t_end = min(t0 + 2, T)
                    if t_end - evac["done"] == 24 or t_end == T:
                        n = t_end - evac["done"]
                        nc.scalar.copy(
                            out=outbuf[:, evac["done"] * 2 * O : t_end * 2 * O],
                            in_=outacc[:, 0 : n * 2 * O],
                        )
                        evac["done"] = t_end

                # ---- prologue: layer-1 for pair 0 ----
                for q in range(2):
                    if q < NPAIR:
                        dma_xb(q)
                xs0 = split_x(0)
                t1c = mm1(0, xs0)
                lif1(0, t1c)

                for p in range(NPAIR):
                    t2a = mm2_half(p, 0)
                    lif2_half(p, 0, t2a)
                    if p + 2 < NPAIR:
                        dma_xb(p + 2)
                    if p + 1 < NPAIR:
                        xs = split_x(p + 1)
                        t1n = mm1(p + 1, xs)
                    t2b = mm2_half(p, 1)
                    lif2_half(p, 1, t2b)
                    # lif1(p+1) must come after both mm2 halves of pair p:
                    # it overwrites the spk1 slots those matmuls read.
                    if p + 1 < NPAIR:
                        lif1(p + 1, t1n)
                    t34 = mm3(p)
                    lif3(p, t34)
                    mm4(p, t34)
                    lif4(p, t34)
                    record(p)

            if debug:
                nc.sync.dma_start(out=dbg["mem1_out"][:], in_=mem1[:])
                nc.sync.dma_start(out=dbg["mem2_out"][:], in_=mem2[:])
                nc.sync.dma_start(out=dbg["mem3_out"][:], in_=mem3[:])
                nc.sync.dma_start(out=dbg["spk1_out"][:], in_=spk1[:].bitcast(F32))
                nc.sync.dma_start(out=dbg["spk2_out"][:], in_=spk2[:].bitcast(F32))
            # ---- final DMA out ----
            ob = outbuf[:].rearrange("b (t x) -> b t x", x=2 * O)
            nc.sync.dma_start(
                out=spk_o[:].rearrange("t b o -> b t o"), in_=ob[:, :, 0:O]
            )
            nc.sync.dma_start(
                out=mem_o[:].rearrange("t b o -> b t o"), in_=ob[:, :, O : 2 * O]
            )

    fix_multi_waits(nc)
    F32R = prev_r
    return nc


_NC_CACHE = {}


def _get_nc(T=T_FULL):
    if T not in _NC_CACHE:
        _NC_CACHE[T] = build_nc(T)
    return _NC_CACHE[T]


def run_cores(inputs, T=T_FULL, n_cores=NCORES, **kw):
    """Run on the first n_cores with batch n_cores*128; returns (spk, mem)."""
    nc = _get_nc(T)
    eye = np.eye(128, dtype=np.float32)
    base = {k: np.asarray(inputs[k], np.float32)
            for k in ("W1", "b1", "W2", "b2", "W3", "b3", "W4", "b4")}
    base["eye"] = eye
    x = np.asarray(inputs["x"], np.float32)
    in_maps = [dict(base, x=x[c * B : (c + 1) * B]) for c in range(n_cores)]
    res = run_bass_kernel_spmd(nc, in_maps, list(range(n_cores)), **kw)
    run_cores.last_result = res
    spk = np.concatenate([res.results[c]["spk_out"] for c in range(n_cores)], axis=1)
    mem = np.concatenate([res.results[c]["mem_out"] for c in range(n_cores)], axis=1)
    return spk, mem


def kernel(x, W1, b1, W2, b2, W3, b3, W4, b4):
    spk, mem = run_cores(
        dict(x=x, W1=W1, b1=b1, W2=W2, b2=b2, W3=W3, b3=b3, W4=W4, b4=b4)
    )
    return spk, mem



# revision 25
# speedup vs baseline: 1.0045x; 1.0045x over previous
"""Trainium2 Bass kernel for nn_AudNet (4-layer LIF SNN, 81-step scan).

Strategy (per core, batch 128 of 1024; data-parallel over 8 cores):
  - Layout: features on partitions, batch on the free dim.
  - Timesteps processed in pairs so every matmul has free dim 256, where
    fp32r runs at 1 cycle/row on the PE.
  - All weights split into fp32r hi + lo terms (residual ~2^-26), so the
    matmuls match fp32 numerics to ~1e-8.  x is split hi/lo too (3-term
    layer 1; the hi/lo split of x is exact).
  - LIF: reset(t) == spk(t-1), so
        mem = 0.95*mem + (cur + b)        (A: scalar_tensor_tensor, PSUM in)
        mem = mem - spk_prev              (B: tensor_sub, in place)
        spk = mem > 1                     (C: tensor_scalar is_gt -> fp32r)
    Biases ride inside the matmuls via constant-one rows in padding
    partitions of the stationary tiles.
  - Outputs (spk4/mem4, [10,128] per step) are PE-transposed into a PSUM
    accumulator and evacuated to SBUF every 24 steps; two strided DMAs
    write the [81,128,10] DRAM outputs.
"""

import numpy as np

import concourse.bass as bass
import concourse.mybir as mybir
import concourse.tile as tile
from concourse.bass_utils import run_bass_kernel_spmd

F32 = mybir.dt.float32
# Reduced dtype for all matmul operands.  fp16 (11-bit mantissa) hi/lo
# 2-term splits carry ~2^-24 residual -- near-fp32 -- while the PE
# streams fp16 moving data at 1 cycle/row (fp32r runs at ~2) and fp16
# stationary tiles get Fast Weight Load (fp32r cannot).
F32R = mybir.dt.float16
GT = mybir.AluOpType.is_gt
MULT = mybir.AluOpType.mult
ADD = mybir.AluOpType.add

T_FULL = 81
F = 129          # input features per step
H = 1000         # hidden width
HP = 1024        # padded hidden width (8 chunks of 128)
HL = 20          # layer-3 width
O = 10           # output width
B = 128          # batch per core
NCORES = 8
BETA = 0.95
TH = 1.0


def fix_multi_waits(nc, limit=1):
    """walrus codegen rejects >1 sem wait on most instructions; spill
    extras onto standalone EventSemaphore instructions in front."""
    ev = 0
    for bb in nc.main_func.blocks:
        out = []
        for ins in bb.instructions:
            si = ins.sync_info
            if si is not None and len(si.on_wait) > limit:
                waits = list(si.on_wait)
                extra, keep = waits[:-limit], waits[-limit:]
                for w in extra:
                    e = mybir.InstEventSemaphore(name=f"evw_{ev}", ins=[], outs=[])
                    ev += 1
                    e.engine = ins.engine
                    e.sync_info = mybir.SyncInfo(on_wait=[w], on_update=[])
                    out.append(e)
                ins.sync_info = mybir.SyncInfo(on_wait=keep, on_update=list(si.on_update))
            out.append(ins)
        bb.instructions = out


def build_nc(T=T_FULL, debug=False, rdt=None):
    global F32R
    prev_r = F32R
    if rdt is not None:
        F32R = rdt
    NPAIR = (T + 1) // 2
    last_odd = (T % 2) == 1  # final pair has only one real step

    nc = bass.Bass()
    _four = mybir.dt.size(F32R) == 4

    def rd(ap):
        """view a reduced-dtype AP as f32 for DVE arithmetic"""
        return ap.bitcast(F32) if _four else ap

    x_d = nc.declare_dram_parameter("x", [B, F * T_FULL], F32, isOutput=False)
    W1_d = nc.declare_dram_parameter("W1", [H, F], F32, isOutput=False)
    b1_d = nc.declare_dram_parameter("b1", [H], F32, isOutput=False)
    W2_d = nc.declare_dram_parameter("W2", [H, H], F32, isOutput=False)
    b2_d = nc.declare_dram_parameter("b2", [H], F32, isOutput=False)
    W3_d = nc.declare_dram_parameter("W3", [HL, H], F32, isOutput=False)
    b3_d = nc.declare_dram_parameter("b3", [HL], F32, isOutput=False)
    W4_d = nc.declare_dram_parameter("W4", [O, HL], F32, isOutput=False)
    b4_d = nc.declare_dram_parameter("b4", [O], F32, isOutput=False)
    eye_d = nc.declare_dram_parameter("eye", [128, 128], F32, isOutput=False)
    if debug:
        dbg = {
            "mem1_out": nc.declare_dram_parameter("mem1_out", [128, HP], F32, isOutput=True),
            "mem2_out": nc.declare_dram_parameter("mem2_out", [128, HP], F32, isOutput=True),
            "mem3_out": nc.declare_dram_parameter("mem3_out", [HL, B], F32, isOutput=True),
            "spk1_out": nc.declare_dram_parameter("spk1_out", [128, 2 * HP], F32, isOutput=True),
            "spk2_out": nc.declare_dram_parameter("spk2_out", [128, 2 * HP], F32, isOutput=True),
        }
    spk_o = nc.declare_dram_parameter("spk_out", [T, B, O], F32, isOutput=True)
    mem_o = nc.declare_dram_parameter("mem_out", [T, B, O], F32, isOutput=True)

    with tile.TileContext(nc) as tc:
        with tc.tile_pool(name="persist", bufs=1) as pp:
            # ---- persistent SBUF tiles ----
            eye = pp.tile([128, 128], F32, tag="eye")
            xT = pp.tile([128, 128 * T_FULL], F32, tag="xT")      # [f<128, b*T+t]
            w1h = pp.tile([128, HP], F32R, tag="w1h")
            w1l = pp.tile([128, HP], F32R, tag="w1l")
            w1bh = pp.tile([2, HP], F32R, tag="w1bh")             # row0 f=128, row1 bias
            w1bl = pp.tile([2, HP], F32R, tag="w1bl")
            w2h = [pp.tile([128, HP], F32R, tag=f"w2h{c}", name=f"w2h{c}") for c in range(8)]
            w2l = [pp.tile([128, HP], F32R, tag=f"w2l{c}", name=f"w2l{c}") for c in range(8)]
            w3h = pp.tile([128, 8 * HL], F32R, tag="w3h")
            w3l = pp.tile([128, 8 * HL], F32R, tag="w3l")
            w4h = pp.tile([HL + 1, O], F32R, tag="w4h")
            w4l = pp.tile([HL + 1, O], F32R, tag="w4l")
            mem1 = pp.tile([128, HP], F32, tag="mem1")
            mem2 = pp.tile([128, HP], F32, tag="mem2")
            mem3 = pp.tile([HL, B], F32, tag="mem3")
            mem4 = pp.tile([O, 2 * B], F32, tag="mem4")
            spk1 = pp.tile([128, 2 * HP], F32R, tag="spk1")       # slot-major
            spk2 = pp.tile([128, 2 * HP], F32R, tag="spk2")
            spk3 = pp.tile([HL + 1, 2 * B], F32R, tag="spk3")     # row HL = ones
            spk4 = pp.tile([O, 2 * B], F32, tag="spk4")
            outbuf = pp.tile([128, T * 2 * O], F32, tag="outbuf")
            ones_f = pp.tile([1, 256], F32, tag="ones_f")
            ones_r = pp.tile([1, 256], F32R, tag="ones_r")
            # layer-1 currents staged in SBUF (two pair slots) so the PE
            # never waits on lif1's PSUM reads (WAR on the old t1 tile)
            cur1 = pp.tile([128, 2 * 2048], F32, tag="cur1")

            nc.sync.dma_start(out=eye[:], in_=eye_d[:])

            # ================= SETUP =================
            with (
                tc.tile_pool(name="setup_sb", bufs=1) as sp,
                tc.tile_pool(name="setup_ps", bufs=4, space="PSUM") as spp,
            ):
                # bias splits (hi/lo in fp32r), kept in partition 0
                def bias_split(b_dram, n, tagbase):
                    bs = sp.tile([1, n], F32, tag=f"{tagbase}s")
                    nc.sync.dma_start(out=bs[:], in_=b_dram[:].rearrange("(a n) -> a n", a=1))
                    bh = sp.tile([1, n], F32R, tag=f"{tagbase}h")
                    bl = sp.tile([1, n], F32R, tag=f"{tagbase}l")
                    nc.vector.tensor_copy(out=bh[:], in_=bs[:])
                    nc.vector.tensor_sub(bl[:], bs[:], rd(bh[:]))
                    return bh, bl

                b1h, b1l = bias_split(b1_d, H, "b1")
                b2h, b2l = bias_split(b2_d, H, "b2")
                b3h, b3l = bias_split(b3_d, HL, "b3")
                b4h, b4l = bias_split(b4_d, O, "b4")

                # zero-init all weight tiles (padding regions stay 0)
                for tl in [w1h, w1l, w1bh, w1bl, w3h, w3l, w4h, w4l] + w2h + w2l:
                    nc.vector.memset(rd(tl[:]), 0.0)

                def evac_split(psum_ap, hi_ap, lo_ap):
                    nc.vector.tensor_copy(out=hi_ap, in_=psum_ap)
                    nc.vector.tensor_sub(lo_ap, psum_ap, rd(hi_ap))

                # ---- W2 ----
                for mc in range(8):
                    mh = 128 if mc < 7 else H - 7 * 128
                    ws = sp.tile([128, H], F32, tag="w2s", bufs=2)
                    nc.sync.dma_start(out=ws[:mh, :], in_=W2_d[mc * 128 : mc * 128 + mh, :])
                    for c in range(8):
                        kw = 128 if c < 7 else H - 7 * 128
                        pt = spp.tile([128, 128], F32, tag="tp")
                        nc.tensor.transpose(
                            pt[:kw, :mh], ws[:mh, c * 128 : c * 128 + kw], eye[:mh, :mh]
                        )
                        evac_split(
                            pt[:kw, :mh],
                            w2h[c][:kw, mc * 128 : mc * 128 + mh],
                            w2l[c][:kw, mc * 128 : mc * 128 + mh],
                        )
                # bias rows: k-chunk 7, partition 104 (feature 1000)
                nc.sync.dma_start(out=w2h[7][104:105, 0:H], in_=b2h[:])
                nc.sync.dma_start(out=w2l[7][104:105, 0:H], in_=b2l[:])

                # ---- W1 ----
                for mc in range(8):
                    mh = 128 if mc < 7 else H - 7 * 128
                    ws = sp.tile([128, F], F32, tag="w1s")
                    nc.sync.dma_start(out=ws[:mh, :], in_=W1_d[mc * 128 : mc * 128 + mh, :])
                    pt = spp.tile([128, 128], F32, tag="tp")
                    nc.tensor.transpose(pt[:128, :mh], ws[:mh, 0:128], eye[:mh, :mh])
                    evac_split(
                        pt[:128, :mh],
                        w1h[:, mc * 128 : mc * 128 + mh],
                        w1l[:, mc * 128 : mc * 128 + mh],
                    )
                    pt2 = spp.tile([128, 128], F32, tag="tp2")
                    nc.tensor.transpose(pt2[:1, :mh], ws[:mh, 128:129], eye[:mh, :mh])
                    evac_split(
                        pt2[:1, :mh],
                        w1bh[0:1, mc * 128 : mc * 128 + mh],
                        w1bl[0:1, mc * 128 : mc * 128 + mh],
                    )
                nc.sync.dma_start(out=w1bh[1:2, 0:H], in_=b1h[:])
                nc.sync.dma_start(out=w1bl[1:2, 0:H], in_=b1l[:])

                # ---- W3 ----
                w3s = sp.tile([HL, H], F32, tag="w3s")
                nc.sync.dma_start(out=w3s[:], in_=W3_d[:])
                for c in range(8):
                    kw = 128 if c < 7 else H - 7 * 128
                    pt = spp.tile([128, 128], F32, tag="tp")
                    nc.tensor.transpose(
                        pt[:kw, :HL], w3s[:, c * 128 : c * 128 + kw], eye[:HL, :HL]
                    )
                    evac_split(
                        pt[:kw, :HL],
                        w3h[:kw, c * HL : (c + 1) * HL],
                        w3l[:kw, c * HL : (c + 1) * HL],
                    )
                nc.sync.dma_start(out=w3h[104:105, 7 * HL : 8 * HL], in_=b3h[:])
                nc.sync.dma_start(out=w3l[104:105, 7 * HL : 8 * HL], in_=b3l[:])

                # ---- W4 ----
                w4s = sp.tile([O, HL], F32, tag="w4s")
                nc.sync.dma_start(out=w4s[:], in_=W4_d[:])
                pt = spp.tile([128, 128], F32, tag="tp")
                nc.tensor.transpose(pt[:HL, :O], w4s[:, :], eye[:O, :O])
                evac_split(pt[:HL, :O], w4h[:HL, :], w4l[:HL, :])
                nc.sync.dma_start(out=w4h[HL : HL + 1, :], in_=b4h[:])
                nc.sync.dma_start(out=w4l[HL : HL + 1, :], in_=b4l[:])

                # ---- x (transposed load: [f, b, t]) ----
                xv = x_d[:].rearrange("b (f t) -> f b t", t=T_FULL)
                nc.sync.dma_start(
                    out=xT[:].rearrange("p (b t) -> p b t", t=T_FULL),
                    in_=xv[0:128, :, :],
                )

                # ---- state init ----
                for tl in [mem1, mem2, mem3, mem4, spk4]:
                    nc.vector.memset(tl[:], 0.0)
                nc.vector.memset(rd(spk1[:]), 0.0)
                nc.vector.memset(rd(spk2[:]), 0.0)
                nc.vector.memset(rd(spk3[:]), 0.0)
                # constant-one bias rhs rows (both slots).  Compute
                # engines need 32-aligned partition starts, so write these
                # single rows via DMA from a ones tile.
                nc.vector.memset(ones_f[:], 1.0)
                nc.vector.tensor_copy(out=ones_r[:], in_=ones_f[:])
                s1w = spk1[:].rearrange("p (s c b) -> p s c b", s=2, b=B)
                s2w = spk2[:].rearrange("p (s c b) -> p s c b", s=2, b=B)
                ones_v = ones_r[:].rearrange("p (s b) -> p s b", s=2)
                nc.sync.dma_start(out=s1w[104:105, :, 7, :], in_=ones_v)
                nc.sync.dma_start(out=s2w[104:105, :, 7, :], in_=ones_v)
                nc.sync.dma_start(out=spk3[HL : HL + 1, :], in_=ones_r[:])

            # ================= SCAN =================
            with (
                tc.tile_pool(name="xs", bufs=2) as xsp,
                tc.tile_pool(name="xb", bufs=3) as xbp,
                tc.tile_pool(name="pl1", bufs=2, space="PSUM") as pl1,
                tc.tile_pool(name="pl2", bufs=1, space="PSUM") as pl2,
                tc.tile_pool(name="pl34", bufs=1, space="PSUM") as pl34,
                tc.tile_pool(name="pout", bufs=1, space="PSUM") as pout,
            ):
                outacc = pout.tile([128, 24 * 2 * O], F32, tag="outacc")
                evac = {"done": 0}

                xT_v = xT[:].rearrange("p (b t) -> p t b", t=T_FULL)
                xb_dram = x_d[:].rearrange("b (f t) -> f t b", t=T_FULL)[128:129]
                s1_v = spk1[:].rearrange("p (s c b) -> p s c b", s=2, b=B)
                s2_v = spk2[:].rearrange("p (s c b) -> p s c b", s=2, b=B)
                m1_v = mem1[:].rearrange("p (c b) -> p c b", b=B)
                m2_v = mem2[:].rearrange("p (c b) -> p c b", b=B)

                xb_tiles = {}

                def dma_xb(p):
                    """prefetch the f=128 feature row for pair p; this DMA is
                    a slow strided gather, so it is issued pairs ahead."""
                    t0 = 2 * p
                    nt = 1 if (last_odd and p == NPAIR - 1) else 2
                    xbr = xbp.tile([2, 256], F32, tag="xbr")
                    xbr_w = xbr[:].rearrange("p (t b) -> p t b", b=B)
                    nc.sync.dma_start(
                        out=xbr_w[0:1, 0:nt, :], in_=xb_dram[:, t0 : t0 + nt, :]
                    )
                    if p < 3:
                        nc.sync.dma_start(out=xbr[1:2, :], in_=ones_f[:])
                    xb_tiles[p] = xbr

                def split_x(p):
                    """fp16 hi/lo split of the x slice for pair p (t-major
                    pair columns); last odd pair duplicates its single step."""
                    t0 = 2 * p
                    dup = last_odd and p == NPAIR - 1
                    nt = 1 if dup else 2
                    src = xT_v[:, t0 : t0 + nt, :]
                    xh = xsp.tile([128, 256], F32R, tag="xh")
                    xl = xsp.tile([128, 256], F32R, tag="xl")
                    xbh = xsp.tile([2, 256], F32R, tag="xbh")
                    xbl = xsp.tile([2, 256], F32R, tag="xbl")
                    xbr = xb_tiles.pop(p)
                    xbr_w = xbr[:].rearrange("p (t b) -> p t b", b=B)
                    if dup:
                        nc.vector.tensor_copy(
                            out=xbr_w[0:1, 1:2, :], in_=xbr_w[0:1, 0:1, :]
                        )
                    srcb = xbr_w[:, 0:2, :]
                    xh_w = xh[:].rearrange("p (t b) -> p t b", b=B)
                    xl_w = xl[:].rearrange("p (t b) -> p t b", b=B)
                    xbh_w = xbh[:].rearrange("p (t b) -> p t b", b=B)
                    xbl_w = xbl[:].rearrange("p (t b) -> p t b", b=B)
                    for s in range(2 if dup else 1):
                        dst = slice(s, s + nt) if dup else slice(0, 2)
                        nc.scalar.copy(out=xh_w[:, dst, :], in_=src)
                        nc.vector.tensor_sub(
                            xl_w[:, dst, :], src, rd(xh_w[:, dst, :])
                        )
                    nc.scalar.copy(out=xbh_w[:], in_=srcb)
                    nc.vector.tensor_sub(xbl_w[:], srcb, rd(xbh_w[:]))
                    return xh, xl, xbh, xbl

                c1_all = cur1[:].rearrange(
                    "p (k c s b) -> p k s c b", k=2, s=2, b=B
                )

                def mm1(p, xs):
                    xh, xl, xbh, xbl = xs
                    base = (p % 2) * 2048
                    for g in range(4):
                        tp = pl1.tile([128, 512], F32, tag="l1")
                        for mi in range(2):
                            mc = 2 * g + mi
                            out = tp[:, mi * 256 : (mi + 1) * 256]
                            ms = slice(mc * 128, (mc + 1) * 128)
                            terms = [
                                (w1h[:, ms], xh[:]),
                                (w1bh[:, ms], xbh[:]),
                                (w1h[:, ms], xl[:]),
                                (w1bh[:, ms], xbl[:]),
                                (w1l[:, ms], xh[:]),
                                (w1bl[:, ms], xbh[:]),
                            ]
                            for i, (lhsT, rhs) in enumerate(terms):
                                nc.tensor.matmul(
                                    out, lhsT=lhsT, rhs=rhs,
                                    start=(i == 0), stop=(i == len(terms) - 1),
                                )
                        nc.scalar.copy(
                            out=cur1[:, base + g * 512 : base + (g + 1) * 512],
                            in_=tp[:],
                        )

                def lif1(p, t1):
                    t0 = 2 * p
                    c1v = c1_all[:, p % 2]
                    for s in range(2):
                        t = t0 + s
                        if t >= T:
                            break
                        sl, slp = t % 2, (t - 1) % 2
                        nc.vector.scalar_tensor_tensor(
                            out=m1_v[:, :, :], in0=m1_v[:, :, :], scalar=BETA,
                            in1=c1v[:, s, :, :], op0=MULT, op1=ADD,
                        )
                        nc.vector.tensor_sub(
                            mem1[:], mem1[:],
                            rd(spk1[:, slp * HP : (slp + 1) * HP]),
                        )
                        nc.vector.tensor_scalar(
                            out=s1_v[:, sl, 0:7, :], in0=m1_v[:, 0:7, :],
                            scalar1=TH, scalar2=None, op0=GT,
                        )
                        nc.vector.tensor_scalar(
                            out=s1_v[0:104, sl, 7, :], in0=m1_v[0:104, 7, :],
                            scalar1=TH, scalar2=None, op0=GT,
                        )

                def mm2_half(p, half):
                    t2 = pl2.tile([128, 1024], F32, tag="l2")
                    for mi in range(4):
                        mc = 4 * half + mi
                        out = t2[:, mi * 256 : (mi + 1) * 256]
                        n = 0
                        for wsp in (w2h, w2l):
                            for c in range(8):
                                nc.tensor.matmul(
                                    out,
                                    lhsT=wsp[c][:, mc * 128 : (mc + 1) * 128],
                                    rhs=s1_v[:, :, c, :],
                                    start=(n == 0), stop=(n == 15),
                                )
                                n += 1
                    return t2

                def lif2_half(p, half, t2):
                    t0 = 2 * p
                    c2v = t2[:].rearrange("p (c s b) -> p s c b", s=2, b=B)
                    c0 = 4 * half
                    for s in range(2):
                        t = t0 + s
                        if t >= T:
                            break
                        sl, slp = t % 2, (t - 1) % 2
                        nc.vector.scalar_tensor_tensor(
                            out=m2_v[:, c0 : c0 + 4, :], in0=m2_v[:, c0 : c0 + 4, :],
                            scalar=BETA, in1=c2v[:, s, :, :], op0=MULT, op1=ADD,
                        )
                        nc.vector.tensor_sub(
                            mem2[:, c0 * B : (c0 + 4) * B],
                            mem2[:, c0 * B : (c0 + 4) * B],
                            rd(spk2[:, slp * HP + c0 * B : slp * HP + (c0 + 4) * B]),
                        )
                        if half == 0:
                            nc.vector.tensor_scalar(
                                out=s2_v[:, sl, 0:4, :], in0=m2_v[:, 0:4, :],
                                scalar1=TH, scalar2=None, op0=GT,
                            )
                        else:
                            nc.vector.tensor_scalar(
                                out=s2_v[:, sl, 4:7, :], in0=m2_v[:, 4:7, :],
                                scalar1=TH, scalar2=None, op0=GT,
                            )
                            nc.vector.tensor_scalar(
                                out=s2_v[0:104, sl, 7, :], in0=m2_v[0:104, 7, :],
                                scalar1=TH, scalar2=None, op0=GT,
                            )

                def mm3(p):
                    t34 = pl34.tile([32, 512], F32, tag="l34")
                    out = t34[0:HL, 0:256]
                    n = 0
                    for wsp in (w3h, w3l):
                        for c in range(8):
                            nc.tensor.matmul(
                                out,
                                lhsT=wsp[:, c * HL : (c + 1) * HL],
                                rhs=s2_v[:, :, c, :],
                                start=(n == 0), stop=(n == 15),
                            )
                            n += 1
                    return t34

                def lif3(p, t34):
                    t0 = 2 * p
                    for s in range(2):
                        t = t0 + s
                        if t >= T:
                            break
                        sl, slp = t % 2, (t - 1) % 2
                        nc.vector.scalar_tensor_tensor(
                            out=mem3[:], in0=mem3[:], scalar=BETA,
                            in1=t34[0:HL, s * B : (s + 1) * B], op0=MULT, op1=ADD,
                        )
                        nc.vector.tensor_sub(
                            mem3[:], mem3[:],
                            rd(spk3[0:HL, slp * B : (slp + 1) * B]),
                        )
                        nc.vector.tensor_scalar(
                            out=spk3[0:HL, sl * B : (sl + 1) * B], in0=mem3[:],
                            scalar1=TH, scalar2=None, op0=GT,
                        )

                def mm4(p, t34):
                    out = t34[0:O, 256:512]
                    nc.tensor.matmul(out, lhsT=w4h[:], rhs=spk3[:], start=True, stop=False)
                    nc.tensor.matmul(out, lhsT=w4l[:], rhs=spk3[:], start=False, stop=True)

                def lif4(p, t34):
                    t0 = 2 * p
                    for s in range(2):
                        t = t0 + s
                        if t >= T:
                            break
                        sl, slp = t % 2, (t - 1) % 2
                        nc.vector.scalar_tensor_tensor(
                            out=mem4[:, sl * B : (sl + 1) * B],
                            in0=mem4[:, slp * B : (slp + 1) * B], scalar=BETA,
                            in1=t34[0:O, 256 + s * B : 256 + (s + 1) * B],
                            op0=MULT, op1=ADD,
                        )
                        nc.vector.tensor_sub(
                            mem4[:, sl * B : (sl + 1) * B],
                            mem4[:, sl * B : (sl + 1) * B],
                            spk4[:, slp * B : (slp + 1) * B],
                        )
                        nc.vector.tensor_scalar(
                            out=spk4[:, sl * B : (sl + 1) * B],
                            in0=mem4[:, sl * B : (sl + 1) * B],
                            scalar1=TH, scalar2=None, op0=GT,
                        )

                def record(p):
                    t0 = 2 * p
                    for s in range(2):
                        t = t0 + s
                        if t >= T:
                            break
                        sl = t % 2
                        w = t - evac["done"]
                        nc.tensor.transpose(
                            outacc[:, w * 2 * O : w * 2 * O + O],
                            spk4[:, sl * B : (sl + 1) * B],
                            eye[:O, :O],
                        )
                        nc.tensor.transpose(
                            outacc[:, w * 2 * O + O : (w + 1) * 2 * O],
                            mem4[:, sl * B : (sl + 1) * B],
                            eye[:O, :O],
                        )
                    t_end = min(t0 + 2, T)
                    if t_end - evac["done"] == 24 or t_end == T:
                        n = t_end - evac["done"]
                        nc.scalar.copy(
                            out=outbuf[:, evac["done"] * 2 * O : t_end * 2 * O],
                            in_=outacc[:, 0 : n * 2 * O],
                        )
                        evac["done"] = t_end

                # ---- prologue: layer-1 for pair 0 ----
                for q in range(2):
                    if q < NPAIR:
                        dma_xb(q)
                xs0 = split_x(0)
                t1c = mm1(0, xs0)
                lif1(0, t1c)

                for p in range(NPAIR):
                    t2a = mm2_half(p, 0)
                    lif2_half(p, 0, t2a)
                    if p + 2 < NPAIR:
                        dma_xb(p + 2)
                    if p + 1 < NPAIR:
                        xs = split_x(p + 1)
                        t1n = mm1(p + 1, xs)
                    t2b = mm2_half(p, 1)
                    lif2_half(p, 1, t2b)
                    # lif1(p+1) must come after both mm2 halves of pair p:
                    # it overwrites the spk1 slots those matmuls read.
                    if p + 1 < NPAIR:
                        lif1(p + 1, t1n)
                    t34 = mm3(p)
                    lif3(p, t34)
                    mm4(p, t34)
                    lif4(p, t34)
                    record(p)

            if debug:
                nc.sync.dma_start(out=dbg["mem1_out"][:], in_=mem1[:])
                nc.sync.dma_start(out=dbg["mem2_out"][:], in_=mem2[:])
                nc.sync.dma_start(out=dbg["mem3_out"][:], in_=mem3[:])
                nc.sync.dma_start(out=dbg["spk1_out"][:], in_=spk1[:].bitcast(F32))
                nc.sync.dma_start(out=dbg["spk2_out"][:], in_=spk2[:].bitcast(F32))
            # ---- final DMA out ----
            ob = outbuf[:].rearrange("b (t x) -> b t x", x=2 * O)
            nc.sync.dma_start(
                out=spk_o[:].rearrange("t b o -> b t o"), in_=ob[:, :, 0:O]
            )
            nc.sync.dma_start(
                out=mem_o[:].rearrange("t b o -> b t o"), in_=ob[:, :, O : 2 * O]
            )

    fix_multi_waits(nc)
    F32R = prev_r
    return nc


_NC_CACHE = {}


def _get_nc(T=T_FULL):
    if T not in _NC_CACHE:
        _NC_CACHE[T] = build_nc(T)
    return _NC_CACHE[T]


def run_cores(inputs, T=T_FULL, n_cores=NCORES, **kw):
    """Run on the first n_cores with batch n_cores*128; returns (spk, mem)."""
    nc = _get_nc(T)
    eye = np.eye(128, dtype=np.float32)
    base = {k: np.asarray(inputs[k], np.float32)
            for k in ("W1", "b1", "W2", "b2", "W3", "b3", "W4", "b4")}
    base["eye"] = eye
    x = np.asarray(inputs["x"], np.float32)
    in_maps = [dict(base, x=x[c * B : (c + 1) * B]) for c in range(n_cores)]
    res = run_bass_kernel_spmd(nc, in_maps, list(range(n_cores)), **kw)
    run_cores.last_result = res
    spk = np.concatenate([res.results[c]["spk_out"] for c in range(n_cores)], axis=1)
    mem = np.concatenate([res.results[c]["mem_out"] for c in range(n_cores)], axis=1)
    return spk, mem


def kernel(x, W1, b1, W2, b2, W3, b3, W4, b4):
    spk, mem = run_cores(
        dict(x=x, W1=W1, b1=b1, W2=W2, b2=b2, W3=W3, b3=b3, W4=W4, b4=b4)
    )
    return spk, mem



# revision 28
# speedup vs baseline: 1.1986x; 1.1933x over previous
"""Trainium2 Bass kernel for nn_AudNet (4-layer LIF SNN, 81-step scan).

Strategy (per core, batch 128 of 1024; data-parallel over 8 cores):
  - Layout: features on partitions, batch on the free dim.
  - Timesteps processed in pairs so every matmul has free dim 256, where
    fp32r runs at 1 cycle/row on the PE.
  - All weights split into fp32r hi + lo terms (residual ~2^-26), so the
    matmuls match fp32 numerics to ~1e-8.  x is split hi/lo too (3-term
    layer 1; the hi/lo split of x is exact).
  - LIF: reset(t) == spk(t-1), so
        mem = 0.95*mem + (cur + b)        (A: scalar_tensor_tensor, PSUM in)
        mem = mem - spk_prev              (B: tensor_sub, in place)
        spk = mem > 1                     (C: tensor_scalar is_gt -> fp32r)
    Biases ride inside the matmuls via constant-one rows in padding
    partitions of the stationary tiles.
  - Outputs (spk4/mem4, [10,128] per step) are PE-transposed into a PSUM
    accumulator and evacuated to SBUF every 24 steps; two strided DMAs
    write the [81,128,10] DRAM outputs.
"""

import numpy as np

import concourse.bass as bass
import concourse.mybir as mybir
import concourse.tile as tile
from concourse.bass_utils import run_bass_kernel_spmd

F32 = mybir.dt.float32
F32R = mybir.dt.float16
GT = mybir.AluOpType.is_gt
MULT = mybir.AluOpType.mult
ADD = mybir.AluOpType.add

T_FULL = 81
F = 129          # input features per step
H = 1000         # hidden width
HP = 1024        # padded hidden width (8 chunks of 128)
HL = 20          # layer-3 width
O = 10           # output width
B = 128          # batch per core
NCORES = 8
BETA = 0.95
TH = 1.0


def fix_multi_waits(nc, limit=1):
    """walrus codegen rejects >1 sem wait on most instructions; spill
    extras onto standalone EventSemaphore instructions in front."""
    ev = 0
    for bb in nc.main_func.blocks:
        out = []
        for ins in bb.instructions:
            si = ins.sync_info
            if si is not None and len(si.on_wait) > limit:
                waits = list(si.on_wait)
                extra, keep = waits[:-limit], waits[-limit:]
                for w in extra:
                    e = mybir.InstEventSemaphore(name=f"evw_{ev}", ins=[], outs=[])
                    ev += 1
                    e.engine = ins.engine
                    e.sync_info = mybir.SyncInfo(on_wait=[w], on_update=[])
                    out.append(e)
                ins.sync_info = mybir.SyncInfo(on_wait=keep, on_update=list(si.on_update))
            out.append(ins)
        bb.instructions = out


def build_nc(T=T_FULL, debug=False, rdt=None):
    global F32R
    prev_r = F32R
    if rdt is not None:
        F32R = rdt
    NPAIR = (T + 1) // 2
    last_odd = (T % 2) == 1  # final pair has only one real step

    nc = bass.Bass()
    _four = mybir.dt.size(F32R) == 4

    def rd(ap):
        """view a reduced-dtype AP as f32 for DVE arithmetic"""
        return ap.bitcast(F32) if _four else ap

    x_d = nc.declare_dram_parameter("x", [B, F * T_FULL], F32, isOutput=False)
    W1_d = nc.declare_dram_parameter("W1", [H, F], F32, isOutput=False)
    b1_d = nc.declare_dram_parameter("b1", [H], F32, isOutput=False)
    W2_d = nc.declare_dram_parameter("W2", [H, H], F32, isOutput=False)
    b2_d = nc.declare_dram_parameter("b2", [H], F32, isOutput=False)
    W3_d = nc.declare_dram_parameter("W3", [HL, H], F32, isOutput=False)
    b3_d = nc.declare_dram_parameter("b3", [HL], F32, isOutput=False)
    W4_d = nc.declare_dram_parameter("W4", [O, HL], F32, isOutput=False)
    b4_d = nc.declare_dram_parameter("b4", [O], F32, isOutput=False)
    eye_d = nc.declare_dram_parameter("eye", [128, 128], F32, isOutput=False)
    if debug:
        dbg = {
            "mem1_out": nc.declare_dram_parameter("mem1_out", [128, HP], F32, isOutput=True),
            "mem2_out": nc.declare_dram_parameter("mem2_out", [128, HP], F32, isOutput=True),
            "mem3_out": nc.declare_dram_parameter("mem3_out", [HL, B], F32, isOutput=True),
            "spk1_out": nc.declare_dram_parameter("spk1_out", [128, 2 * HP], F32, isOutput=True),
            "spk2_out": nc.declare_dram_parameter("spk2_out", [128, 2 * HP], F32, isOutput=True),
        }
    spk_o = nc.declare_dram_parameter("spk_out", [T, B, O], F32, isOutput=True)
    mem_o = nc.declare_dram_parameter("mem_out", [T, B, O], F32, isOutput=True)

    with tile.TileContext(nc) as tc:
        with tc.tile_pool(name="persist", bufs=1) as pp:
            # ---- persistent SBUF tiles ----
            eye = pp.tile([128, 128], F32, tag="eye")
            xT = pp.tile([128, 128 * T_FULL], F32, tag="xT")      # [f<128, b*T+t]
            w1h = pp.tile([128, HP], F32R, tag="w1h")
            w1l = pp.tile([128, HP], F32R, tag="w1l")
            w1bh = pp.tile([2, HP], F32R, tag="w1bh")             # row0 f=128, row1 bias
            w1bl = pp.tile([2, HP], F32R, tag="w1bl")
            w1b5 = pp.tile([5, HP], F32R, tag="w1b5")
            w2h = [pp.tile([128, HP], F32R, tag=f"w2h{c}", name=f"w2h{c}") for c in range(8)]
            w2l = [pp.tile([128, HP], F32R, tag=f"w2l{c}", name=f"w2l{c}") for c in range(8)]
            # hi cols [0:20], zero pad [20:32], lo cols [32:52] per chunk --
            # one matmul then computes hi and lo stacked on the partition
            # axis of the output (lo lands 32-aligned for the DVE adds)
            w3m = pp.tile([128, 8 * 52], F32R, tag="w3m")
            w4m = pp.tile([HL + 1, 42], F32R, tag="w4m")
            mem1 = pp.tile([128, HP], F32, tag="mem1")
            mem2 = pp.tile([128, HP], F32, tag="mem2")
            mem3 = pp.tile([HL, B], F32, tag="mem3")
            mem4 = pp.tile([O, 2 * B], F32, tag="mem4")
            spk1 = pp.tile([128, 2 * HP], F32R, tag="spk1")       # slot-major
            spk2 = pp.tile([128, 2 * HP], F32R, tag="spk2")
            spk3 = pp.tile([HL + 1, 2 * B], F32R, tag="spk3")     # row HL = ones
            spk4 = pp.tile([O, 2 * B], F32, tag="spk4")
            outbuf = pp.tile([128, T * 2 * O], F32, tag="outbuf")
            ones_f = pp.tile([1, 256], F32, tag="ones_f")
            ones_r = pp.tile([1, 256], F32R, tag="ones_r")

            nc.sync.dma_start(out=eye[:], in_=eye_d[:])

            # ================= SETUP =================
            with (
                tc.tile_pool(name="setup_sb", bufs=1) as sp,
                tc.tile_pool(name="setup_ps", bufs=4, space="PSUM") as spp,
            ):
                # bias splits (hi/lo in fp32r), kept in partition 0
                def bias_split(b_dram, n, tagbase):
                    bs = sp.tile([1, n], F32, tag=f"{tagbase}s")
                    nc.sync.dma_start(out=bs[:], in_=b_dram[:].rearrange("(a n) -> a n", a=1))
                    bh = sp.tile([1, n], F32R, tag=f"{tagbase}h")
                    bl = sp.tile([1, n], F32R, tag=f"{tagbase}l")
                    nc.vector.tensor_copy(out=bh[:], in_=bs[:])
                    nc.vector.tensor_sub(bl[:], bs[:], rd(bh[:]))
                    return bh, bl

                b1h, b1l = bias_split(b1_d, H, "b1")
                b2h, b2l = bias_split(b2_d, H, "b2")
                b3h, b3l = bias_split(b3_d, HL, "b3")
                b4h, b4l = bias_split(b4_d, O, "b4")

                # zero-init all weight tiles (padding regions stay 0)
                for tl in [w1h, w1l, w1bh, w1bl, w1b5, w3m, w4m] + w2h + w2l:
                    nc.vector.memset(rd(tl[:]), 0.0)

                def evac_split(psum_ap, hi_ap, lo_ap):
                    nc.vector.tensor_copy(out=hi_ap, in_=psum_ap)
                    nc.vector.tensor_sub(lo_ap, psum_ap, rd(hi_ap))

                # ---- W2 ----
                for mc in range(8):
                    mh = 128 if mc < 7 else H - 7 * 128
                    ws = sp.tile([128, H], F32, tag="w2s", bufs=2)
                    nc.sync.dma_start(out=ws[:mh, :], in_=W2_d[mc * 128 : mc * 128 + mh, :])
                    for c in range(8):
                        kw = 128 if c < 7 else H - 7 * 128
                        pt = spp.tile([128, 128], F32, tag="tp")
                        nc.tensor.transpose(
                            pt[:kw, :mh], ws[:mh, c * 128 : c * 128 + kw], eye[:mh, :mh]
                        )
                        evac_split(
                            pt[:kw, :mh],
                            w2h[c][:kw, mc * 128 : mc * 128 + mh],
                            w2l[c][:kw, mc * 128 : mc * 128 + mh],
                        )
                # bias rows: k-chunk 7, partition 104 (feature 1000)
                nc.sync.dma_start(out=w2h[7][104:105, 0:H], in_=b2h[:])
                nc.sync.dma_start(out=w2l[7][104:105, 0:H], in_=b2l[:])

                # ---- W1 ----
                for mc in range(8):
                    mh = 128 if mc < 7 else H - 7 * 128
                    ws = sp.tile([128, F], F32, tag="w1s")
                    nc.sync.dma_start(out=ws[:mh, :], in_=W1_d[mc * 128 : mc * 128 + mh, :])
                    pt = spp.tile([128, 128], F32, tag="tp")
                    nc.tensor.transpose(pt[:128, :mh], ws[:mh, 0:128], eye[:mh, :mh])
                    evac_split(
                        pt[:128, :mh],
                        w1h[:, mc * 128 : mc * 128 + mh],
                        w1l[:, mc * 128 : mc * 128 + mh],
                    )
                    pt2 = spp.tile([128, 128], F32, tag="tp2")
                    nc.tensor.transpose(pt2[:1, :mh], ws[:mh, 128:129], eye[:mh, :mh])
                    evac_split(
                        pt2[:1, :mh],
                        w1bh[0:1, mc * 128 : mc * 128 + mh],
                        w1bl[0:1, mc * 128 : mc * 128 + mh],
                    )
                nc.sync.dma_start(out=w1bh[1:2, 0:H], in_=b1h[:])
                nc.sync.dma_start(out=w1bl[1:2, 0:H], in_=b1l[:])
                # merged rank-5 stationary for the f=128 + bias terms:
                # rows pair with X5 = [xf_h, ones, xf_l, ones, xf_h]
                nc.sync.dma_start(out=w1b5[0:2, :], in_=w1bh[0:2, :])
                nc.sync.dma_start(out=w1b5[2:3, :], in_=w1bh[0:1, :])
                nc.sync.dma_start(out=w1b5[3:4, :], in_=w1bl[1:2, :])
                nc.sync.dma_start(out=w1b5[4:5, :], in_=w1bl[0:1, :])

                # ---- W3 ----
                w3s = sp.tile([HL, H], F32, tag="w3s")
                nc.sync.dma_start(out=w3s[:], in_=W3_d[:])
                for c in range(8):
                    kw = 128 if c < 7 else H - 7 * 128
                    pt = spp.tile([128, 128], F32, tag="tp")
                    nc.tensor.transpose(
                        pt[:kw, :HL], w3s[:, c * 128 : c * 128 + kw], eye[:HL, :HL]
                    )
                    evac_split(
                        pt[:kw, :HL],
                        w3m[:kw, c * 52 : c * 52 + HL],
                        w3m[:kw, c * 52 + 32 : c * 52 + 32 + HL],
                    )
                nc.sync.dma_start(out=w3m[104:105, 7 * 52 : 7 * 52 + HL], in_=b3h[:])
                nc.sync.dma_start(
                    out=w3m[104:105, 7 * 52 + 32 : 7 * 52 + 32 + HL], in_=b3l[:]
                )

                # ---- W4 ----
                w4s = sp.tile([O, HL], F32, tag="w4s")
                nc.sync.dma_start(out=w4s[:], in_=W4_d[:])
                pt = spp.tile([128, 128], F32, tag="tp")
                nc.tensor.transpose(pt[:HL, :O], w4s[:, :], eye[:O, :O])
                evac_split(pt[:HL, :O], w4m[:HL, 0:O], w4m[:HL, 32 : 32 + O])
                nc.sync.dma_start(out=w4m[HL : HL + 1, 0:O], in_=b4h[:])
                nc.sync.dma_start(out=w4m[HL : HL + 1, 32 : 32 + O], in_=b4l[:])

                # ---- x (transposed load: [f, b, t]) ----
                xv = x_d[:].rearrange("b (f t) -> f b t", t=T_FULL)
                nc.sync.dma_start(
                    out=xT[:].rearrange("p (b t) -> p b t", t=T_FULL),
                    in_=xv[0:128, :, :],
                )

                # ---- state init ----
                for tl in [mem1, mem2, mem3, mem4, spk4]:
                    nc.vector.memset(tl[:], 0.0)
                nc.vector.memset(rd(spk1[:]), 0.0)
                nc.vector.memset(rd(spk2[:]), 0.0)
                nc.vector.memset(rd(spk3[:]), 0.0)
                # constant-one bias rhs rows (both slots).  Compute
                # engines need 32-aligned partition starts, so write these
                # single rows via DMA from a ones tile.
                nc.vector.memset(ones_f[:], 1.0)
                nc.vector.tensor_copy(out=ones_r[:], in_=ones_f[:])
                s1w = spk1[:].rearrange("p (s c b) -> p s c b", s=2, b=B)
                s2w = spk2[:].rearrange("p (s c b) -> p s c b", s=2, b=B)
                ones_v = ones_r[:].rearrange("p (s b) -> p s b", s=2)
                nc.sync.dma_start(out=s1w[104:105, :, 7, :], in_=ones_v)
                nc.sync.dma_start(out=s2w[104:105, :, 7, :], in_=ones_v)
                nc.sync.dma_start(out=spk3[HL : HL + 1, :], in_=ones_r[:])

            # ================= SCAN =================
            with (
                tc.tile_pool(name="xs", bufs=2) as xsp,
                tc.tile_pool(name="pl1", bufs=1, space="PSUM") as pl1,
                tc.tile_pool(name="pl2", bufs=1, space="PSUM") as pl2,
                tc.tile_pool(name="pl34", bufs=1, space="PSUM") as pl34,
                tc.tile_pool(name="pout", bufs=1, space="PSUM") as pout,
            ):
                outacc = pout.tile([128, 24 * 2 * O], F32, tag="outacc")
                evac = {"done": 0}

                xT_v = xT[:].rearrange("p (b t) -> p t b", t=T_FULL)
                xb_dram = x_d[:].rearrange("b (f t) -> f t b", t=T_FULL)[128:129]
                s1_v = spk1[:].rearrange("p (s c b) -> p s c b", s=2, b=B)
                s2_v = spk2[:].rearrange("p (s c b) -> p s c b", s=2, b=B)
                m1_v = mem1[:].rearrange("p (c b) -> p c b", b=B)
                m2_v = mem2[:].rearrange("p (c b) -> p c b", b=B)

                def split_x(p):
                    """fp32r hi/lo split of the x slice for pair p (t-major
                    pair columns); last odd pair duplicates its single step."""
                    t0 = 2 * p
                    dup = last_odd and p == NPAIR - 1
                    nt = 1 if dup else 2
                    src = xT_v[:, t0 : t0 + nt, :]
                    xh = xsp.tile([128, 256], F32R, tag="xh")
                    xl = xsp.tile([128, 256], F32R, tag="xl")
                    x5 = xsp.tile([5, 256], F32R, tag="x5")
                    xbl = xsp.tile([2, 256], F32R, tag="xbl")
                    xbr = xsp.tile([2, 256], F32, tag="xbr")
                    xbr_w = xbr[:].rearrange("p (t b) -> p t b", b=B)
                    nc.sync.dma_start(
                        out=xbr_w[0:1, 0:nt, :], in_=xb_dram[:, t0 : t0 + nt, :]
                    )
                    if dup:
                        nc.vector.tensor_copy(
                            out=xbr_w[0:1, 1:2, :], in_=xbr_w[0:1, 0:1, :]
                        )
                    if p < 2:
                        nc.sync.dma_start(out=xbr[1:2, :], in_=ones_f[:])
                    srcb = xbr_w[:, 0:2, :]
                    xh_w = xh[:].rearrange("p (t b) -> p t b", b=B)
                    xl_w = xl[:].rearrange("p (t b) -> p t b", b=B)
                    for s in range(2 if dup else 1):
                        dst = slice(s, s + nt) if dup else slice(0, 2)
                        nc.scalar.copy(out=xh_w[:, dst, :], in_=src)
                        nc.vector.tensor_sub(
                            xl_w[:, dst, :], src, rd(xh_w[:, dst, :])
                        )
                    nc.scalar.copy(out=x5[0:2, :].rearrange(
                        "p (t b) -> p t b", b=B), in_=srcb)
                    nc.vector.tensor_sub(
                        xbl[:].rearrange("p (t b) -> p t b", b=B),
                        srcb, x5[0:2, :].rearrange("p (t b) -> p t b", b=B),
                    )
                    # assemble rows 2..4 = [xf_l, ones, xf_h] (DMA: compute
                    # engines cannot write non-32-aligned partition starts)
                    nc.sync.dma_start(out=x5[2:3, :], in_=xbl[0:1, :])
                    if p < 2:
                        nc.sync.dma_start(out=x5[3:4, :], in_=x5[1:2, :])
                    nc.sync.dma_start(out=x5[4:5, :], in_=x5[0:1, :])
                    return xh, xl, x5

                def mm1(p, xs):
                    xh, xl, x5 = xs
                    t1 = pl1.tile([128, 2048], F32, tag="l1")
                    for mc in range(8):
                        out = t1[:, mc * 256 : (mc + 1) * 256]
                        ms = slice(mc * 128, (mc + 1) * 128)
                        terms = [
                            (w1h[:, ms], xh[:]),
                            (w1l[:, ms], xh[:]),
                            (w1h[:, ms], xl[:]),
                        ]
                        for i, (lhsT, rhs) in enumerate(terms):
                            nc.tensor.matmul(
                                out, lhsT=lhsT, rhs=rhs,
                                start=(i == 0), stop=False,
                            )
                        nc.tensor.matmul(
                            out, lhsT=w1b5[:, ms], rhs=x5[:],
                            start=False, stop=True,
                        )
                    return t1

                def lif1(p, t1):
                    t0 = 2 * p
                    c1v = t1[:].rearrange("p (c s b) -> p s c b", s=2, b=B)
                    for s in range(2):
                        t = t0 + s
                        if t >= T:
                            break
                        sl, slp = t % 2, (t - 1) % 2
                        nc.vector.scalar_tensor_tensor(
                            out=m1_v[:, :, :], in0=m1_v[:, :, :], scalar=BETA,
                            in1=c1v[:, s, :, :], op0=MULT, op1=ADD,
                        )
                        nc.vector.tensor_sub(
                            mem1[:], mem1[:],
                            rd(spk1[:, slp * HP : (slp + 1) * HP]),
                        )
                        nc.vector.tensor_scalar(
                            out=s1_v[:, sl, 0:7, :], in0=m1_v[:, 0:7, :],
                            scalar1=TH, scalar2=None, op0=GT,
                        )
                        nc.vector.tensor_scalar(
                            out=s1_v[0:104, sl, 7, :], in0=m1_v[0:104, 7, :],
                            scalar1=TH, scalar2=None, op0=GT,
                        )

                def mm2_half(p, half):
                    t2 = pl2.tile([128, 1024], F32, tag="l2")
                    for mi in range(4):
                        mc = 4 * half + mi
                        out = t2[:, mi * 256 : (mi + 1) * 256]
                        n = 0
                        for wsp in (w2h, w2l):
                            for c in range(8):
                                nc.tensor.matmul(
                                    out,
                                    lhsT=wsp[c][:, mc * 128 : (mc + 1) * 128],
                                    rhs=s1_v[:, :, c, :],
                                    start=(n == 0), stop=(n == 15),
                                )
                                n += 1
                    return t2

                def lif2_half(p, half, t2):
                    t0 = 2 * p
                    c2v = t2[:].rearrange("p (c s b) -> p s c b", s=2, b=B)
                    c0 = 4 * half
                    for s in range(2):
                        t = t0 + s
                        if t >= T:
                            break
                        sl, slp = t % 2, (t - 1) % 2
                        nc.vector.scalar_tensor_tensor(
                            out=m2_v[:, c0 : c0 + 4, :], in0=m2_v[:, c0 : c0 + 4, :],
                            scalar=BETA, in1=c2v[:, s, :, :], op0=MULT, op1=ADD,
                        )
                        nc.vector.tensor_sub(
                            mem2[:, c0 * B : (c0 + 4) * B],
                            mem2[:, c0 * B : (c0 + 4) * B],
                            rd(spk2[:, slp * HP + c0 * B : slp * HP + (c0 + 4) * B]),
                        )
                        if half == 0:
                            nc.vector.tensor_scalar(
                                out=s2_v[:, sl, 0:4, :], in0=m2_v[:, 0:4, :],
                                scalar1=TH, scalar2=None, op0=GT,
                            )
                        else:
                            nc.vector.tensor_scalar(
                                out=s2_v[:, sl, 4:7, :], in0=m2_v[:, 4:7, :],
                                scalar1=TH, scalar2=None, op0=GT,
                            )
                            nc.vector.tensor_scalar(
                                out=s2_v[0:104, sl, 7, :], in0=m2_v[0:104, 7, :],
                                scalar1=TH, scalar2=None, op0=GT,
                            )

                def mm3(p):
                    t34 = pl34.tile([52, 512], F32, tag="l34")
                    out = t34[0:52, 0:256]
                    for c in range(8):
                        nc.tensor.matmul(
                            out,
                            lhsT=w3m[:, c * 52 : (c + 1) * 52],
                            rhs=s2_v[:, :, c, :],
                            start=(c == 0), stop=(c == 7),
                        )
                    return t34

                def lif3(p, t34):
                    t0 = 2 * p
                    for s in range(2):
                        t = t0 + s
                        if t >= T:
                            break
                        sl, slp = t % 2, (t - 1) % 2
                        nc.vector.scalar_tensor_tensor(
                            out=mem3[:], in0=mem3[:], scalar=BETA,
                            in1=t34[0:HL, s * B : (s + 1) * B], op0=MULT, op1=ADD,
                        )
                        nc.vector.tensor_add(
                            mem3[:], mem3[:],
                            t34[32 : 32 + HL, s * B : (s + 1) * B],
                        )
                        nc.vector.tensor_sub(
                            mem3[:], mem3[:],
                            rd(spk3[0:HL, slp * B : (slp + 1) * B]),
                        )
                        nc.vector.tensor_scalar(
                            out=spk3[0:HL, sl * B : (sl + 1) * B], in0=mem3[:],
                            scalar1=TH, scalar2=None, op0=GT,
                        )

                def mm4(p, t34):
                    out = t34[0:42, 256:512]
                    nc.tensor.matmul(out, lhsT=w4m[:], rhs=spk3[:], start=True, stop=True)

                def lif4(p, t34):
                    t0 = 2 * p
                    for s in range(2):
                        t = t0 + s
                        if t >= T:
                            break
                        sl, slp = t % 2, (t - 1) % 2
                        nc.vector.scalar_tensor_tensor(
                            out=mem4[:, sl * B : (sl + 1) * B],
                            in0=mem4[:, slp * B : (slp + 1) * B], scalar=BETA,
                            in1=t34[0:O, 256 + s * B : 256 + (s + 1) * B],
                            op0=MULT, op1=ADD,
                        )
                        nc.vector.tensor_add(
                            mem4[:, sl * B : (sl + 1) * B],
                            mem4[:, sl * B : (sl + 1) * B],
                            t34[32 : 32 + O, 256 + s * B : 256 + (s + 1) * B],
                        )
                        nc.vector.tensor_sub(
                            mem4[:, sl * B : (sl + 1) * B],
                            mem4[:, sl * B : (sl + 1) * B],
                            spk4[:, slp * B : (slp + 1) * B],
                        )
                        nc.vector.tensor_scalar(
                            out=spk4[:, sl * B : (sl + 1) * B],
                            in0=mem4[:, sl * B : (sl + 1) * B],
                            scalar1=TH, scalar2=None, op0=GT,
                        )

                def record(p):
                    t0 = 2 * p
                    for s in range(2):
                        t = t0 + s
                        if t >= T:
                            break
                        sl = t % 2
                        w = t - evac["done"]
                        nc.tensor.transpose(
                            outacc[:, w * 2 * O : w * 2 * O + O],
                            spk4[:, sl * B : (sl + 1) * B],
                            eye[:O, :O],
                        )
                        nc.tensor.transpose(
                            outacc[:, w * 2 * O + O : (w + 1) * 2 * O],
                            mem4[:, sl * B : (sl + 1) * B],
                            eye[:O, :O],
                        )
                    t_end = min(t0 + 2, T)
                    if t_end - evac["done"] == 24 or t_end == T:
                        n = t_end - evac["done"]
                        nc.scalar.copy(
                            out=outbuf[:, evac["done"] * 2 * O : t_end * 2 * O],
                            in_=outacc[:, 0 : n * 2 * O],
                        )
                        evac["done"] = t_end

                # ---- prologue: layer-1 for pair 0 ----
                xs0 = split_x(0)
                t1c = mm1(0, xs0)
                lif1(0, t1c)

                for p in range(NPAIR):
                    t2a = mm2_half(p, 0)
                    lif2_half(p, 0, t2a)
                    if p + 1 < NPAIR:
                        xs = split_x(p + 1)
                        t1n = mm1(p + 1, xs)
                    t2b = mm2_half(p, 1)
                    lif2_half(p, 1, t2b)
                    # lif1(p+1) must come after both mm2 halves of pair p:
                    # it overwrites the spk1 slots those matmuls read.
                    if p + 1 < NPAIR:
                        lif1(p + 1, t1n)
                    t34 = mm3(p)
                    lif3(p, t34)
                    mm4(p, t34)
                    lif4(p, t34)
                    record(p)

            if debug:
                nc.sync.dma_start(out=dbg["mem1_out"][:], in_=mem1[:])
                nc.sync.dma_start(out=dbg["mem2_out"][:], in_=mem2[:])
                nc.sync.dma_start(out=dbg["mem3_out"][:], in_=mem3[:])
                nc.sync.dma_start(out=dbg["spk1_out"][:], in_=spk1[:].bitcast(F32))
                nc.sync.dma_start(out=dbg["spk2_out"][:], in_=spk2[:].bitcast(F32))
            # ---- final DMA out ----
            ob = outbuf[:].rearrange("b (t x) -> b t x", x=2 * O)
            nc.sync.dma_start(
                out=spk_o[:].rearrange("t b o -> b t o"), in_=ob[:, :, 0:O]
            )
            nc.sync.dma_start(
                out=mem_o[:].rearrange("t b o -> b t o"), in_=ob[:, :, O : 2 * O]
            )

    fix_multi_waits(nc)
    F32R = prev_r
    return nc


_NC_CACHE = {}


def _get_nc(T=T_FULL):
    if T not in _NC_CACHE:
        _NC_CACHE[T] = build_nc(T)
    return _NC_CACHE[T]


def run_cores(inputs, T=T_FULL, n_cores=NCORES, **kw):
    """Run on the first n_cores with batch n_cores*128; returns (spk, mem)."""
    nc = _get_nc(T)
    eye = np.eye(128, dtype=np.float32)
    base = {k: np.asarray(inputs[k], np.float32)
            for k in ("W1", "b1", "W2", "b2", "W3", "b3", "W4", "b4")}
    base["eye"] = eye
    x = np.asarray(inputs["x"], np.float32)
    in_maps = [dict(base, x=x[c * B : (c + 1) * B]) for c in range(n_cores)]
    res = run_bass_kernel_spmd(nc, in_maps, list(range(n_cores)), **kw)
    run_cores.last_result = res
    spk = np.concatenate([res.results[c]["spk_out"] for c in range(n_cores)], axis=1)
    mem = np.concatenate([res.results[c]["mem_out"] for c in range(n_cores)], axis=1)
    return spk, mem


def kernel(x, W1, b1, W2, b2, W3, b3, W4, b4):
    spk, mem = run_cores(
        dict(x=x, W1=W1, b1=b1, W2=W2, b2=b2, W3=W3, b3=b3, W4=W4, b4=b4)
    )
    return spk, mem



# revision 31
# speedup vs baseline: 1.2403x; 1.0348x over previous
"""Trainium2 Bass kernel for nn_AudNet (4-layer LIF SNN, 81-step scan).

Strategy (per core, batch 128 of 1024; data-parallel over 8 cores):
  - Layout: features on partitions, batch on the free dim.
  - Timesteps processed in pairs so every matmul has free dim 256, where
    fp32r runs at 1 cycle/row on the PE.
  - All weights split into fp32r hi + lo terms (residual ~2^-26), so the
    matmuls match fp32 numerics to ~1e-8.  x is split hi/lo too (3-term
    layer 1; the hi/lo split of x is exact).
  - LIF: reset(t) == spk(t-1), so
        mem = 0.95*mem + (cur + b)        (A: scalar_tensor_tensor, PSUM in)
        mem = mem - spk_prev              (B: tensor_sub, in place)
        spk = mem > 1                     (C: tensor_scalar is_gt -> fp32r)
    Biases ride inside the matmuls via constant-one rows in padding
    partitions of the stationary tiles.
  - Outputs (spk4/mem4, [10,128] per step) are PE-transposed into a PSUM
    accumulator and evacuated to SBUF every 24 steps; two strided DMAs
    write the [81,128,10] DRAM outputs.
"""

import numpy as np

import concourse.bass as bass
import concourse.mybir as mybir
import concourse.tile as tile
from concourse.bass_utils import run_bass_kernel_spmd

F32 = mybir.dt.float32
F32R = mybir.dt.float16
GT = mybir.AluOpType.is_gt
MULT = mybir.AluOpType.mult
ADD = mybir.AluOpType.add

T_FULL = 81
F = 129          # input features per step
H = 1000         # hidden width
HP = 1024        # padded hidden width (8 chunks of 128)
HL = 20          # layer-3 width
O = 10           # output width
B = 128          # batch per core
NCORES = 8
BETA = 0.95
TH = 1.0


def fix_multi_waits(nc, limit=1):
    """walrus codegen rejects >1 sem wait on most instructions; spill
    extras onto standalone EventSemaphore instructions in front."""
    ev = 0
    for bb in nc.main_func.blocks:
        out = []
        for ins in bb.instructions:
            si = ins.sync_info
            if si is not None and len(si.on_wait) > limit:
                waits = list(si.on_wait)
                extra, keep = waits[:-limit], waits[-limit:]
                for w in extra:
                    e = mybir.InstEventSemaphore(name=f"evw_{ev}", ins=[], outs=[])
                    ev += 1
                    e.engine = ins.engine
                    e.sync_info = mybir.SyncInfo(on_wait=[w], on_update=[])
                    out.append(e)
                ins.sync_info = mybir.SyncInfo(on_wait=keep, on_update=list(si.on_update))
            out.append(ins)
        bb.instructions = out


def build_nc(T=T_FULL, debug=False, rdt=None):
    global F32R
    prev_r = F32R
    if rdt is not None:
        F32R = rdt
    NPAIR = (T + 1) // 2
    last_odd = (T % 2) == 1  # final pair has only one real step

    nc = bass.Bass()
    _four = mybir.dt.size(F32R) == 4

    def rd(ap):
        """view a reduced-dtype AP as f32 for DVE arithmetic"""
        return ap.bitcast(F32) if _four else ap

    x_d = nc.declare_dram_parameter("x", [B, F * T_FULL], F32, isOutput=False)
    W1_d = nc.declare_dram_parameter("W1", [H, F], F32, isOutput=False)
    b1_d = nc.declare_dram_parameter("b1", [H], F32, isOutput=False)
    W2_d = nc.declare_dram_parameter("W2", [H, H], F32, isOutput=False)
    b2_d = nc.declare_dram_parameter("b2", [H], F32, isOutput=False)
    W3_d = nc.declare_dram_parameter("W3", [HL, H], F32, isOutput=False)
    b3_d = nc.declare_dram_parameter("b3", [HL], F32, isOutput=False)
    W4_d = nc.declare_dram_parameter("W4", [O, HL], F32, isOutput=False)
    b4_d = nc.declare_dram_parameter("b4", [O], F32, isOutput=False)
    eye_d = nc.declare_dram_parameter("eye", [128, 128], F32, isOutput=False)
    if debug:
        dbg = {
            "mem1_out": nc.declare_dram_parameter("mem1_out", [128, HP], F32, isOutput=True),
            "mem2_out": nc.declare_dram_parameter("mem2_out", [128, HP], F32, isOutput=True),
            "mem3_out": nc.declare_dram_parameter("mem3_out", [HL, B], F32, isOutput=True),
            "spk1_out": nc.declare_dram_parameter("spk1_out", [128, 2 * HP], F32, isOutput=True),
            "spk2_out": nc.declare_dram_parameter("spk2_out", [128, 2 * HP], F32, isOutput=True),
        }
    spk_o = nc.declare_dram_parameter("spk_out", [T, B, O], F32, isOutput=True)
    mem_o = nc.declare_dram_parameter("mem_out", [T, B, O], F32, isOutput=True)

    with tile.TileContext(nc) as tc:
        with tc.tile_pool(name="persist", bufs=1) as pp:
            # ---- persistent SBUF tiles ----
            eye = pp.tile([128, 128], F32, tag="eye")
            xT = pp.tile([128, 128 * T_FULL], F32, tag="xT")      # [f<128, b*T+t]
            w1h = pp.tile([128, HP], F32R, tag="w1h")
            w1l = pp.tile([128, HP], F32R, tag="w1l")
            w1bh = pp.tile([2, HP], F32R, tag="w1bh")             # row0 f=128, row1 bias
            w1bl = pp.tile([2, HP], F32R, tag="w1bl")
            w1b5 = pp.tile([5, HP], F32R, tag="w1b5")
            w2h = [pp.tile([128, HP], F32R, tag=f"w2h{c}", name=f"w2h{c}") for c in range(8)]
            w2l = [pp.tile([128, HP], F32R, tag=f"w2l{c}", name=f"w2l{c}") for c in range(8)]
            # hi cols [0:20], zero pad [20:32], lo cols [32:52] per chunk --
            # one matmul then computes hi and lo stacked on the partition
            # axis of the output (lo lands 32-aligned for the DVE adds)
            w3m = pp.tile([128, 8 * 52], F32R, tag="w3m")
            w4m = pp.tile([HL + 1, 42], F32R, tag="w4m")
            mem1 = pp.tile([128, HP], F32, tag="mem1")
            mem2 = pp.tile([128, HP], F32, tag="mem2")
            mem3 = pp.tile([HL, B], F32, tag="mem3")
            mem4 = pp.tile([O, 2 * B], F32, tag="mem4")
            spk1 = pp.tile([128, 2 * HP], F32R, tag="spk1")       # slot-major
            spk2 = pp.tile([128, 2 * HP], F32R, tag="spk2")
            spk3 = pp.tile([HL + 1, 2 * B], F32R, tag="spk3")     # row HL = ones
            spk4 = pp.tile([O, 2 * B], F32, tag="spk4")
            outbuf = pp.tile([128, T * 2 * O], F32, tag="outbuf")
            ones_f = pp.tile([1, 256], F32, tag="ones_f")
            ones_r = pp.tile([1, 256], F32R, tag="ones_r")

            nc.sync.dma_start(out=eye[:], in_=eye_d[:])

            # ================= SETUP =================
            with (
                tc.tile_pool(name="setup_sb", bufs=1) as sp,
                tc.tile_pool(name="setup_ps", bufs=4, space="PSUM") as spp,
            ):
                # ---- issue every DRAM load up front (all independent
                # queues); compute consumers follow in dependency order ----
                xv = x_d[:].rearrange("b (f t) -> f b t", t=T_FULL)
                nc.sync.dma_start(
                    out=xT[:].rearrange("p (b t) -> p b t", t=T_FULL),
                    in_=xv[0:128, :, :],
                )
                w2s = [sp.tile([128, H], F32, tag=f"w2s{mc}", name=f"w2s{mc}") for mc in range(8)]
                for mc in range(8):
                    mh = 128 if mc < 7 else H - 7 * 128
                    nc.sync.dma_start(
                        out=w2s[mc][:mh, :], in_=W2_d[mc * 128 : mc * 128 + mh, :]
                    )
                w1s = sp.tile([128, F * 8], F32, tag="w1s")
                w1sv = w1s[:].rearrange("p (c f) -> p c f", c=8)
                for mc in range(8):
                    mh = 128 if mc < 7 else H - 7 * 128
                    nc.sync.dma_start(
                        out=w1sv[:mh, mc, :], in_=W1_d[mc * 128 : mc * 128 + mh, :]
                    )
                w3s = sp.tile([HL, H], F32, tag="w3s")
                nc.sync.dma_start(out=w3s[:], in_=W3_d[:])
                w4s = sp.tile([O, HL], F32, tag="w4s")
                nc.sync.dma_start(out=w4s[:], in_=W4_d[:])

                def bias_load(b_dram, n, tagbase):
                    bs = sp.tile([1, n], F32, tag=f"{tagbase}s")
                    nc.sync.dma_start(out=bs[:], in_=b_dram[:].rearrange("(a n) -> a n", a=1))
                    return bs

                b1s = bias_load(b1_d, H, "b1")
                b2s = bias_load(b2_d, H, "b2")
                b3s = bias_load(b3_d, HL, "b3")
                b4s = bias_load(b4_d, O, "b4")

                # zero-init all weight tiles (padding regions stay 0)
                for tl in [w1h, w1l, w1bh, w1bl, w1b5, w3m, w4m] + w2h + w2l:
                    nc.vector.memset(rd(tl[:]), 0.0)

                def evac_split(psum_ap, hi_ap, lo_ap):
                    nc.vector.tensor_copy(out=hi_ap, in_=psum_ap)
                    nc.vector.tensor_sub(lo_ap, psum_ap, rd(hi_ap))

                # ---- W2 ----
                for mc in range(8):
                    mh = 128 if mc < 7 else H - 7 * 128
                    ws = w2s[mc]
                    for c in range(8):
                        kw = 128 if c < 7 else H - 7 * 128
                        pt = spp.tile([128, 128], F32, tag="tp")
                        nc.tensor.transpose(
                            pt[:kw, :mh], ws[:mh, c * 128 : c * 128 + kw], eye[:mh, :mh]
                        )
                        evac_split(
                            pt[:kw, :mh],
                            w2h[c][:kw, mc * 128 : mc * 128 + mh],
                            w2l[c][:kw, mc * 128 : mc * 128 + mh],
                        )

                # ---- W1 ----
                for mc in range(8):
                    mh = 128 if mc < 7 else H - 7 * 128
                    ws = w1sv[:, mc, :]
                    pt = spp.tile([128, 128], F32, tag="tp")
                    nc.tensor.transpose(pt[:128, :mh], ws[:mh, 0:128], eye[:mh, :mh])
                    evac_split(
                        pt[:128, :mh],
                        w1h[:, mc * 128 : mc * 128 + mh],
                        w1l[:, mc * 128 : mc * 128 + mh],
                    )
                    pt2 = spp.tile([128, 128], F32, tag="tp2")
                    nc.tensor.transpose(pt2[:1, :mh], ws[:mh, 128:129], eye[:mh, :mh])
                    evac_split(
                        pt2[:1, :mh],
                        w1bh[0:1, mc * 128 : mc * 128 + mh],
                        w1bl[0:1, mc * 128 : mc * 128 + mh],
                    )

                # ---- W3 ----
                for c in range(8):
                    kw = 128 if c < 7 else H - 7 * 128
                    pt = spp.tile([128, 128], F32, tag="tp")
                    nc.tensor.transpose(
                        pt[:kw, :HL], w3s[:, c * 128 : c * 128 + kw], eye[:HL, :HL]
                    )
                    evac_split(
                        pt[:kw, :HL],
                        w3m[:kw, c * 52 : c * 52 + HL],
                        w3m[:kw, c * 52 + 32 : c * 52 + 32 + HL],
                    )

                # ---- W4 ----
                pt = spp.tile([128, 128], F32, tag="tp")
                nc.tensor.transpose(pt[:HL, :O], w4s[:, :], eye[:O, :O])
                evac_split(pt[:HL, :O], w4m[:HL, 0:O], w4m[:HL, 32 : 32 + O])

                # ---- bias hi/lo splits (late: DVE is free by now) ----
                def bias_split(bs, n, tagbase):
                    bh = sp.tile([1, n], F32R, tag=f"{tagbase}h")
                    bl = sp.tile([1, n], F32R, tag=f"{tagbase}l")
                    nc.vector.tensor_copy(out=bh[:], in_=bs[:])
                    nc.vector.tensor_sub(bl[:], bs[:], rd(bh[:]))
                    return bh, bl

                b1h, b1l = bias_split(b1s, H, "b1")
                b2h, b2l = bias_split(b2s, H, "b2")
                b3h, b3l = bias_split(b3s, HL, "b3")
                b4h, b4l = bias_split(b4s, O, "b4")

                nc.sync.dma_start(out=w2h[7][104:105, 0:H], in_=b2h[:])
                nc.sync.dma_start(out=w2l[7][104:105, 0:H], in_=b2l[:])
                nc.sync.dma_start(out=w1bh[1:2, 0:H], in_=b1h[:])
                nc.sync.dma_start(out=w1bl[1:2, 0:H], in_=b1l[:])
                nc.sync.dma_start(out=w1b5[0:2, :], in_=w1bh[0:2, :])
                nc.sync.dma_start(out=w1b5[2:3, :], in_=w1bh[0:1, :])
                nc.sync.dma_start(out=w1b5[3:4, :], in_=w1bl[1:2, :])
                nc.sync.dma_start(out=w1b5[4:5, :], in_=w1bl[0:1, :])
                nc.sync.dma_start(out=w3m[104:105, 7 * 52 : 7 * 52 + HL], in_=b3h[:])
                nc.sync.dma_start(
                    out=w3m[104:105, 7 * 52 + 32 : 7 * 52 + 32 + HL], in_=b3l[:]
                )
                nc.sync.dma_start(out=w4m[HL : HL + 1, 0:O], in_=b4h[:])
                nc.sync.dma_start(out=w4m[HL : HL + 1, 32 : 32 + O], in_=b4l[:])

                # ---- state init ----
                for tl in [mem1, mem2, mem3, mem4, spk4]:
                    nc.vector.memset(tl[:], 0.0)
                nc.vector.memset(rd(spk1[:]), 0.0)
                nc.vector.memset(rd(spk2[:]), 0.0)
                nc.vector.memset(rd(spk3[:]), 0.0)
                # constant-one bias rhs rows (both slots).  Compute
                # engines need 32-aligned partition starts, so write these
                # single rows via DMA from a ones tile.
                nc.vector.memset(ones_f[:], 1.0)
                nc.vector.tensor_copy(out=ones_r[:], in_=ones_f[:])
                s1w = spk1[:].rearrange("p (s c b) -> p s c b", s=2, b=B)
                s2w = spk2[:].rearrange("p (s c b) -> p s c b", s=2, b=B)
                ones_v = ones_r[:].rearrange("p (s b) -> p s b", s=2)
                nc.sync.dma_start(out=s1w[104:105, :, 7, :], in_=ones_v)
                nc.sync.dma_start(out=s2w[104:105, :, 7, :], in_=ones_v)
                nc.sync.dma_start(out=spk3[HL : HL + 1, :], in_=ones_r[:])

            # ================= SCAN =================
            with (
                tc.tile_pool(name="xs", bufs=2) as xsp,
                tc.tile_pool(name="pl1", bufs=1, space="PSUM") as pl1,
                tc.tile_pool(name="pl2", bufs=1, space="PSUM") as pl2,
                tc.tile_pool(name="pl34", bufs=1, space="PSUM") as pl34,
                tc.tile_pool(name="pout", bufs=1, space="PSUM") as pout,
            ):
                outacc = pout.tile([128, 24 * 2 * O], F32, tag="outacc")
                evac = {"done": 0}

                xT_v = xT[:].rearrange("p (b t) -> p t b", t=T_FULL)
                xb_dram = x_d[:].rearrange("b (f t) -> f t b", t=T_FULL)[128:129]
                s1_v = spk1[:].rearrange("p (s c b) -> p s c b", s=2, b=B)
                s2_v = spk2[:].rearrange("p (s c b) -> p s c b", s=2, b=B)
                m1_v = mem1[:].rearrange("p (c b) -> p c b", b=B)
                m2_v = mem2[:].rearrange("p (c b) -> p c b", b=B)

                def split_x(p):
                    """fp32r hi/lo split of the x slice for pair p (t-major
                    pair columns); last odd pair duplicates its single step."""
                    t0 = 2 * p
                    dup = last_odd and p == NPAIR - 1
                    nt = 1 if dup else 2
                    src = xT_v[:, t0 : t0 + nt, :]
                    xh = xsp.tile([128, 256], F32R, tag="xh")
                    xl = xsp.tile([128, 256], F32R, tag="xl")
                    x5 = xsp.tile([5, 256], F32R, tag="x5")
                    xbl = xsp.tile([2, 256], F32R, tag="xbl")
                    xbr = xsp.tile([2, 256], F32, tag="xbr")
                    xbr_w = xbr[:].rearrange("p (t b) -> p t b", b=B)
                    nc.sync.dma_start(
                        out=xbr_w[0:1, 0:nt, :], in_=xb_dram[:, t0 : t0 + nt, :]
                    )
                    if dup:
                        nc.vector.tensor_copy(
                            out=xbr_w[0:1, 1:2, :], in_=xbr_w[0:1, 0:1, :]
                        )
                    if p < 2:
                        nc.sync.dma_start(out=xbr[1:2, :], in_=ones_f[:])
                    srcb = xbr_w[:, 0:2, :]
                    xh_w = xh[:].rearrange("p (t b) -> p t b", b=B)
                    xl_w = xl[:].rearrange("p (t b) -> p t b", b=B)
                    for s in range(2 if dup else 1):
                        dst = slice(s, s + nt) if dup else slice(0, 2)
                        nc.scalar.copy(out=xh_w[:, dst, :], in_=src)
                        nc.vector.tensor_sub(
                            xl_w[:, dst, :], src, rd(xh_w[:, dst, :])
                        )
                    nc.scalar.copy(out=x5[0:2, :].rearrange(
                        "p (t b) -> p t b", b=B), in_=srcb)
                    nc.vector.tensor_sub(
                        xbl[:].rearrange("p (t b) -> p t b", b=B),
                        srcb, x5[0:2, :].rearrange("p (t b) -> p t b", b=B),
                    )
                    # assemble rows 2..4 = [xf_l, ones, xf_h] (DMA: compute
                    # engines cannot write non-32-aligned partition starts)
                    nc.sync.dma_start(out=x5[2:3, :], in_=xbl[0:1, :])
                    if p < 2:
                        nc.sync.dma_start(out=x5[3:4, :], in_=x5[1:2, :])
                    nc.sync.dma_start(out=x5[4:5, :], in_=x5[0:1, :])
                    return xh, xl, x5

                def mm1(p, xs):
                    xh, xl, x5 = xs
                    t1 = pl1.tile([128, 2048], F32, tag="l1")
                    for mc in range(8):
                        out = t1[:, mc * 256 : (mc + 1) * 256]
                        ms = slice(mc * 128, (mc + 1) * 128)
                        terms = [
                            (w1h[:, ms], xh[:]),
                            (w1l[:, ms], xh[:]),
                            (w1h[:, ms], xl[:]),
                        ]
                        for i, (lhsT, rhs) in enumerate(terms):
                            nc.tensor.matmul(
                                out, lhsT=lhsT, rhs=rhs,
                                start=(i == 0), stop=False,
                            )
                        nc.tensor.matmul(
                            out, lhsT=w1b5[:, ms], rhs=x5[:],
                            start=False, stop=True,
                        )
                    return t1

                def lif1(p, t1):
                    t0 = 2 * p
                    c1v = t1[:].rearrange("p (c s b) -> p s c b", s=2, b=B)
                    for s in range(2):
                        t = t0 + s
                        if t >= T:
                            break
                        sl, slp = t % 2, (t - 1) % 2
                        nc.vector.scalar_tensor_tensor(
                            out=m1_v[:, :, :], in0=m1_v[:, :, :], scalar=BETA,
                            in1=c1v[:, s, :, :], op0=MULT, op1=ADD,
                        )
                        nc.vector.tensor_sub(
                            mem1[:], mem1[:],
                            rd(spk1[:, slp * HP : (slp + 1) * HP]),
                        )
                        nc.vector.tensor_scalar(
                            out=s1_v[:, sl, 0:7, :], in0=m1_v[:, 0:7, :],
                            scalar1=TH, scalar2=None, op0=GT,
                        )
                        nc.vector.tensor_scalar(
                            out=s1_v[0:104, sl, 7, :], in0=m1_v[0:104, 7, :],
                            scalar1=TH, scalar2=None, op0=GT,
                        )

                def mm2_half(p, half):
                    t2 = pl2.tile([128, 1024], F32, tag="l2")
                    for mi in range(4):
                        mc = 4 * half + mi
                        out = t2[:, mi * 256 : (mi + 1) * 256]
                        n = 0
                        for wsp in (w2h, w2l):
                            for c in range(8):
                                nc.tensor.matmul(
                                    out,
                                    lhsT=wsp[c][:, mc * 128 : (mc + 1) * 128],
                                    rhs=s1_v[:, :, c, :],
                                    start=(n == 0), stop=(n == 15),
                                )
                                n += 1
                    return t2

                def lif2_half(p, half, t2):
                    t0 = 2 * p
                    c2v = t2[:].rearrange("p (c s b) -> p s c b", s=2, b=B)
                    c0 = 4 * half
                    for s in range(2):
                        t = t0 + s
                        if t >= T:
                            break
                        sl, slp = t % 2, (t - 1) % 2
                        nc.vector.scalar_tensor_tensor(
                            out=m2_v[:, c0 : c0 + 4, :], in0=m2_v[:, c0 : c0 + 4, :],
                            scalar=BETA, in1=c2v[:, s, :, :], op0=MULT, op1=ADD,
                        )
                        nc.vector.tensor_sub(
                            mem2[:, c0 * B : (c0 + 4) * B],
                            mem2[:, c0 * B : (c0 + 4) * B],
                            rd(spk2[:, slp * HP + c0 * B : slp * HP + (c0 + 4) * B]),
                        )
                        if half == 0:
                            nc.vector.tensor_scalar(
                                out=s2_v[:, sl, 0:4, :], in0=m2_v[:, 0:4, :],
                                scalar1=TH, scalar2=None, op0=GT,
                            )
                        else:
                            nc.vector.tensor_scalar(
                                out=s2_v[:, sl, 4:7, :], in0=m2_v[:, 4:7, :],
                                scalar1=TH, scalar2=None, op0=GT,
                            )
                            nc.vector.tensor_scalar(
                                out=s2_v[0:104, sl, 7, :], in0=m2_v[0:104, 7, :],
                                scalar1=TH, scalar2=None, op0=GT,
                            )

                def mm3(p):
                    t34 = pl34.tile([52, 512], F32, tag="l34")
                    out = t34[0:52, 0:256]
                    for c in range(8):
                        nc.tensor.matmul(
                            out,
                            lhsT=w3m[:, c * 52 : (c + 1) * 52],
                            rhs=s2_v[:, :, c, :],
                            start=(c == 0), stop=(c == 7),
                        )
                    return t34

                def lif3(p, t34):
                    t0 = 2 * p
                    for s in range(2):
                        t = t0 + s
                        if t >= T:
                            break
                        sl, slp = t % 2, (t - 1) % 2
                        nc.vector.scalar_tensor_tensor(
                            out=mem3[:], in0=mem3[:], scalar=BETA,
                            in1=t34[0:HL, s * B : (s + 1) * B], op0=MULT, op1=ADD,
                        )
                        nc.vector.tensor_add(
                            mem3[:], mem3[:],
                            t34[32 : 32 + HL, s * B : (s + 1) * B],
                        )
                        nc.vector.tensor_sub(
                            mem3[:], mem3[:],
                            rd(spk3[0:HL, slp * B : (slp + 1) * B]),
                        )
                        nc.vector.tensor_scalar(
                            out=spk3[0:HL, sl * B : (sl + 1) * B], in0=mem3[:],
                            scalar1=TH, scalar2=None, op0=GT,
                        )

                def mm4(p, t34):
                    out = t34[0:42, 256:512]
                    nc.tensor.matmul(out, lhsT=w4m[:], rhs=spk3[:], start=True, stop=True)

                def lif4(p, t34):
                    t0 = 2 * p
                    for s in range(2):
                        t = t0 + s
                        if t >= T:
                            break
                        sl, slp = t % 2, (t - 1) % 2
                        nc.vector.scalar_tensor_tensor(
                            out=mem4[:, sl * B : (sl + 1) * B],
                            in0=mem4[:, slp * B : (slp + 1) * B], scalar=BETA,
                            in1=t34[0:O, 256 + s * B : 256 + (s + 1) * B],
                            op0=MULT, op1=ADD,
                        )
                        nc.vector.tensor_add(
                            mem4[:, sl * B : (sl + 1) * B],
                            mem4[:, sl * B : (sl + 1) * B],
                            t34[32 : 32 + O, 256 + s * B : 256 + (s + 1) * B],
                        )
                        nc.vector.tensor_sub(
                            mem4[:, sl * B : (sl + 1) * B],
                            mem4[:, sl * B : (sl + 1) * B],
                            spk4[:, slp * B : (slp + 1) * B],
                        )
                        nc.vector.tensor_scalar(
                            out=spk4[:, sl * B : (sl + 1) * B],
                            in0=mem4[:, sl * B : (sl + 1) * B],
                            scalar1=TH, scalar2=None, op0=GT,
                        )

                def record(p):
                    t0 = 2 * p
                    for s in range(2):
                        t = t0 + s
                        if t >= T:
                            break
                        sl = t % 2
                        w = t - evac["done"]
                        nc.tensor.transpose(
                            outacc[:, w * 2 * O : w * 2 * O + O],
                            spk4[:, sl * B : (sl + 1) * B],
                            eye[:O, :O],
                        )
                        nc.tensor.transpose(
                            outacc[:, w * 2 * O + O : (w + 1) * 2 * O],
                            mem4[:, sl * B : (sl + 1) * B],
                            eye[:O, :O],
                        )
                    t_end = min(t0 + 2, T)
                    if t_end - evac["done"] == 24 or t_end == T:
                        n = t_end - evac["done"]
                        nc.scalar.copy(
                            out=outbuf[:, evac["done"] * 2 * O : t_end * 2 * O],
                            in_=outacc[:, 0 : n * 2 * O],
                        )
                        # stream this block to DRAM now instead of in one
                        # serial tail DMA after the scan
                        obb = outbuf[:].rearrange("b (t x) -> b t x", x=2 * O)
                        d0, d1 = evac["done"], t_end
                        nc.sync.dma_start(
                            out=spk_o[d0:d1].rearrange("t b o -> b t o"),
                            in_=obb[:, d0:d1, 0:O],
                        )
                        nc.sync.dma_start(
                            out=mem_o[d0:d1].rearrange("t b o -> b t o"),
                            in_=obb[:, d0:d1, O : 2 * O],
                        )
                        evac["done"] = t_end

                # ---- prologue: layer-1 for pair 0 ----
                xs0 = split_x(0)
                t1c = mm1(0, xs0)
                lif1(0, t1c)

                for p in range(NPAIR):
                    t2a = mm2_half(p, 0)
                    lif2_half(p, 0, t2a)
                    if p + 1 < NPAIR:
                        xs = split_x(p + 1)
                        t1n = mm1(p + 1, xs)
                    t2b = mm2_half(p, 1)
                    lif2_half(p, 1, t2b)
                    # lif1(p+1) must come after both mm2 halves of pair p:
                    # it overwrites the spk1 slots those matmuls read.
                    if p + 1 < NPAIR:
                        lif1(p + 1, t1n)
                    t34 = mm3(p)
                    lif3(p, t34)
                    mm4(p, t34)
                    lif4(p, t34)
                    record(p)

            if debug:
                nc.sync.dma_start(out=dbg["mem1_out"][:], in_=mem1[:])
                nc.sync.dma_start(out=dbg["mem2_out"][:], in_=mem2[:])
                nc.sync.dma_start(out=dbg["mem3_out"][:], in_=mem3[:])
                nc.sync.dma_start(out=dbg["spk1_out"][:], in_=spk1[:].bitcast(F32))
                nc.sync.dma_start(out=dbg["spk2_out"][:], in_=spk2[:].bitcast(F32))

    fix_multi_waits(nc)
    F32R = prev_r
    return nc


_NC_CACHE = {}


def _get_nc(T=T_FULL):
    if T not in _NC_CACHE:
        _NC_CACHE[T] = build_nc(T)
    return _NC_CACHE[T]


def run_cores(inputs, T=T_FULL, n_cores=NCORES, **kw):
    """Run on the first n_cores with batch n_cores*128; returns (spk, mem)."""
    nc = _get_nc(T)
    eye = np.eye(128, dtype=np.float32)
    base = {k: np.asarray(inputs[k], np.float32)
            for k in ("W1", "b1", "W2", "b2", "W3", "b3", "W4", "b4")}
    base["eye"] = eye
    x = np.asarray(inputs["x"], np.float32)
    in_maps = [dict(base, x=x[c * B : (c + 1) * B]) for c in range(n_cores)]
    res = run_bass_kernel_spmd(nc, in_maps, list(range(n_cores)), **kw)
    run_cores.last_result = res
    spk = np.concatenate([res.results[c]["spk_out"] for c in range(n_cores)], axis=1)
    mem = np.concatenate([res.results[c]["mem_out"] for c in range(n_cores)], axis=1)
    return spk, mem


def kernel(x, W1, b1, W2, b2, W3, b3, W4, b4):
    spk, mem = run_cores(
        dict(x=x, W1=W1, b1=b1, W2=W2, b2=b2, W3=W3, b3=b3, W4=W4, b4=b4)
    )
    return spk, mem

